# revision 9
# baseline (speedup 1.0000x reference)
"""Trainium2 Bass kernel for AuxiliaryMultiHeadedAttention.

Reference computation (B=4, L=2048, H=256, NH=8, DH=32):
    kb   = split_heads(k_b @ Wb.T + bb)
    corr = (qh @ kh^T + qh @ kb^T) / sqrt(DH) * scale_w[h, q]
    corr = where(mask==0, -1e9, corr);  prob = softmax(corr)
    out  = merge_heads(prob @ vh) @ Ww.T + bw

Kernel strategy (8 NeuronCores):
    Shard (batch, query-half): core c -> batch c//2, queries (c%2)*1024..+1024.
    Each core:
      kT, kbT via bf16 cast + DMA-xbar transpose (no PE transposes)
      keffT = (k + k_b @ Wb.T + bb)^T  [dims, keys]  bf16 (dual QK^T folded)
      qsT   = (q * scale_w/sqrt(DH))^T [dims, queries] bf16
      S^T   = keffT_h^T @ qsT_h  (bf16 MMs, 2 heads row-tiled, fp32 psum;
              row groups alternate with kc parity via 64-row-shifted tile
              copies so LDWEIGHTS overlaps the previous chunk's matmuls)
      P^T   = exp(S^T): split between ACT (exact exp, bf16 out) and DVE
              (Schraudolph: int16(A*x+B) bitcast to bf16, one tensor_scalar)
      PV with lhsT [m|v_h] / [v_h|m] (m = mask: masks both numerator and
              denominator) -> psum rows [den0|O0|O1|den1]
      hidT  = O * recip(den)  (full-partition recip + mul, DMA realign)
      out   = hidT^T @ WwT + bw  (bf16 MMs)
    Host concatenates the 8 [1024, 256] slices.
"""

import sys

if "/opt/trn_rl_repo" not in sys.path:
    sys.path.insert(0, "/opt/trn_rl_repo")

import math

import numpy as np

B, L, H, NH, DH = 4, 2048, 256, 8, 32
LQ = 1024  # queries per core
NCORES = 8
ISQ = 1.0 / math.sqrt(DH)

# Schraudolph exp for bf16 target: bf16bits(exp(x)) ~ int16(A16*x + B16)
A16 = 128.0 / math.log(2.0)
C_OFF = 5.5
B16 = 127.0 * 128.0 - C_OFF
# every DVE_EXCL-th qb1 exp tile goes to ACT instead of DVE (load balance)
DVE_EXCL = 4


def _build():
    import concourse.bass as bass  # noqa: F401
    import concourse.mybir as mybir
    import concourse.tile as tile
    from concourse import bacc

    f32 = mybir.dt.float32
    i32 = mybir.dt.int32
    i16 = mybir.dt.int16
    bf16 = mybir.dt.bfloat16
    Exp = mybir.ActivationFunctionType.Exp
    Alu = mybir.AluOpType

    nc = bacc.Bacc("TRN2", target_bir_lowering=False, debug=False, num_devices=NCORES)

    q_d = nc.dram_tensor("q_s", [LQ, H], f32, kind="ExternalInput")
    k_d = nc.dram_tensor("k_s", [L, H], f32, kind="ExternalInput")
    v_d = nc.dram_tensor("v_s", [L, H], f32, kind="ExternalInput")
    kb_d = nc.dram_tensor("kb_s", [L, H], f32, kind="ExternalInput")
    mask_d = nc.dram_tensor("mask_s", [L], i32, kind="ExternalInput")
    sw_d = nc.dram_tensor("sw_s", [NH, LQ], f32, kind="ExternalInput")
    Wb_d = nc.dram_tensor("Wb", [H, H], f32, kind="ExternalInput")
    bb_d = nc.dram_tensor("bb", [H], f32, kind="ExternalInput")
    Ww_d = nc.dram_tensor("Ww", [H, H], f32, kind="ExternalInput")
    bw_d = nc.dram_tensor("bw", [H], f32, kind="ExternalInput")
    id_d = nc.dram_tensor("ident", [128, 128], f32, kind="ExternalInput")
    out_d = nc.dram_tensor("out", [LQ, H], f32, kind="ExternalOutput")

    copy_flip = [0]

    with tile.TileContext(nc) as tc:
        with (
            tc.tile_pool(name="persist", bufs=1) as pp,
            tc.tile_pool(name="pt", bufs=3) as ptp,
            tc.tile_pool(name="small", bufs=2) as smp,
        ):
            # ---------------- persistent SBUF tensors ----------------
            ident = pp.tile([128, 128], f32, tag="ident")
            nc.sync.dma_start(out=ident, in_=id_d[:, :])
            keffT = [pp.tile([128, L], bf16, tag=f"keffT{g}", name=f"keffT{g}")
                     for g in range(2)]
            keffT2 = [pp.tile([128, L], bf16, tag=f"keffT2_{g}",
                              name=f"keffT2_{g}") for g in range(2)]
            qsT = [pp.tile([128, LQ], bf16, tag=f"qsT{g}", name=f"qsT{g}")
                   for g in range(2)]
            qsT2 = [pp.tile([128, LQ], bf16, tag=f"qsT2_{g}", name=f"qsT2_{g}")
                    for g in range(2)]
            # per (key-chunk, head): [m|v_h] (h even) / [v_h|m] (h odd);
            # m = mask column (masks numerator and denominator)
            vmm = pp.tile([128, 16 * NH * 64], bf16, tag="vmm")
            hidT = [pp.tile([128, LQ], bf16, tag=f"hidT{g}", name=f"hidT{g}")
                    for g in range(2)]
            WwT = [pp.tile([128, H], bf16, tag=f"WwT{g}", name=f"WwT{g}")
                   for g in range(2)]
            ones1b = pp.tile([1, 128], bf16, tag="ones1b")
            nc.vector.memset(ones1b, 1.0)
            bwb = pp.tile([1, H], bf16, tag="bwb")
            sc8 = pp.tile([128, 64], f32, tag="sc8")
            outsb = pp.tile([128, 8 * H], f32, tag="outsb")

            with tc.tile_pool(name="stage", bufs=1) as sp:
                def pcopy(dst, src):
                    # alternate psum->sbuf evacuation between DVE and ACT
                    if copy_flip[0] % 2 == 0:
                        nc.vector.tensor_copy(dst, src)
                    else:
                        nc.scalar.copy(dst, src)
                    copy_flip[0] += 1

                # ---------------- staging loads ----------------
                swt = sp.tile([NH, LQ], f32, tag="swt")
                nc.sync.dma_start(out=swt, in_=sw_d[:, :])
                qraw = sp.tile([128, 8 * H], f32, tag="qraw")
                nc.sync.dma_start(out=qraw.rearrange("p (c e) -> p c e", c=8),
                                  in_=q_d.rearrange("(c p) e -> p c e", p=128))
                wbraw = sp.tile([128, 2 * H], f32, tag="wbraw")
                nc.sync.dma_start(out=wbraw.rearrange("p (c e) -> p c e", c=2),
                                  in_=Wb_d.rearrange("(c p) e -> p c e", p=128))
                # k, k_b: load f32 split by dim-half, cast bf16, xbar-transpose
                kbf = [sp.tile([128, 16 * 128], f32, tag=f"kbf{e}",
                               name=f"kbf{e}") for e in range(2)]
                kf = [sp.tile([128, 16 * 128], f32, tag=f"kf{e}",
                              name=f"kf{e}") for e in range(2)]
                for tiles, dram in ((kbf, kb_d), (kf, k_d)):
                    dv = dram.rearrange("(c p) e -> p c e", p=128)
                    for ec in range(2):
                        tv = tiles[ec].rearrange("p (c f) -> p c f", c=16)
                        for c4 in range(2):
                            nc.sync.dma_start(
                                out=tv[:, c4 * 8:(c4 + 1) * 8, :],
                                in_=dv[:, c4 * 8:(c4 + 1) * 8,
                                       ec * 128:(ec + 1) * 128])
                kb16 = [sp.tile([128, 16 * 128], bf16, tag=f"kb16_{e}",
                                name=f"kb16_{e}") for e in range(2)]
                k16 = [sp.tile([128, 16 * 128], bf16, tag=f"k16_{e}",
                               name=f"k16_{e}") for e in range(2)]
                kbT = [sp.tile([128, L], bf16, tag=f"kbT{e}", name=f"kbT{e}")
                       for e in range(2)]
                kT = [sp.tile([128, L], bf16, tag=f"kT{e}", name=f"kT{e}")
                      for e in range(2)]
                for ec in range(2):
                    nc.vector.tensor_copy(kb16[ec], kbf[ec])
                    nc.sync.dma_start_transpose(
                        out=kbT[ec].rearrange("p (c f) -> p c f", c=16),
                        in_=kb16[ec])
                    nc.vector.tensor_copy(k16[ec], kf[ec])
                    nc.sync.dma_start_transpose(
                        out=kT[ec].rearrange("p (c f) -> p c f", c=16),
                        in_=k16[ec])

                vraw = sp.tile([128, 16 * H], f32, tag="vraw")
                vv = vraw.rearrange("p (c e) -> p c e", c=16)
                dvv = v_d.rearrange("(c p) e -> p c e", p=128)
                for c4 in range(4):
                    nc.sync.dma_start(out=vv[:, c4 * 4:(c4 + 1) * 4, :],
                                      in_=dvv[:, c4 * 4:(c4 + 1) * 4, :])
                m16 = sp.tile([16, 128], i32, tag="m16")
                nc.sync.dma_start(out=m16,
                                  in_=mask_d.rearrange("(c p) -> c p", p=128))
                wwraw = sp.tile([128, 2 * H], f32, tag="wwraw")
                nc.sync.dma_start(out=wwraw.rearrange("p (c e) -> p c e", c=2),
                                  in_=Ww_d.rearrange("(c p) e -> p c e", p=128))
                bbt = sp.tile([1, H], f32, tag="bbt")
                nc.sync.dma_start(out=bbt, in_=bb_d[None, :])
                bbb = sp.tile([1, H], bf16, tag="bbb")
                nc.vector.tensor_copy(bbb, bbt)
                bwt = sp.tile([1, H], f32, tag="bwt")
                nc.sync.dma_start(out=bwt, in_=bw_d[None, :])
                nc.vector.tensor_copy(bwb, bwt)
                oneslb = sp.tile([1, L], bf16, tag="oneslb")
                nc.vector.memset(oneslb, 1.0)
                m16f = sp.tile([16, 128], f32, tag="m16f")
                nc.vector.tensor_copy(m16f, m16)
                maskf = sp.tile([128, 16], f32, tag="maskf")
                WbT = [sp.tile([128, H], bf16, tag=f"WbT{e}", name=f"WbT{e}")
                       for e in range(2)]

                # ---------------- prep: transposes & keff ----------------
                with (
                    tc.tile_pool(name="ptr", bufs=4, space="PSUM") as ptr,
                    tc.tile_pool(name="pkeff", bufs=1, space="PSUM") as pkf,
                ):
                    # mask -> maskf [128, 16]
                    tm = ptr.tile([128, 16], f32, tag="tr")
                    nc.tensor.transpose(tm, m16f, ident[0:16, 0:16])
                    nc.vector.tensor_copy(maskf, tm)

                    # vmm: [m|v]/[v|m] layout; v and m both masked-scaled
                    # vmm6: p, kc, hpair, parity, half(two), d
                    vmm6 = vmm.rearrange("p (c hp par two d) -> p c hp par two d",
                                         c=16, hp=4, par=2, two=2)
                    vraw4 = vraw.rearrange("p (c hp par d) -> p c hp par d",
                                           c=16, hp=4, par=2)
                    for par in range(2):
                        for hp in range(4):
                            nc.vector.scalar_tensor_tensor(
                                out=vmm6[:, :, hp, par, 1 - par, :],
                                in0=vraw4[:, :, hp, par, :], scalar=1.0,
                                in1=maskf[:, :, None].broadcast_to(
                                    [128, 16, 32]),
                                op0=Alu.mult, op1=Alu.mult)
                            nc.scalar.copy(
                                vmm6[:, :, hp, par, par, :],
                                maskf[:, :, None].broadcast_to(
                                    [128, 16, 32]))

                    # scale_w slices -> sc8 [128, 8 per q-chunk]
                    for mq in range(8):
                        t = ptr.tile([128, 8], f32, tag="tr", name="t")
                        nc.tensor.transpose(t, swt[:, mq * 128:(mq + 1) * 128],
                                            ident[0:NH, 0:NH])
                        nc.vector.tensor_copy(sc8[:, mq * 8:(mq + 1) * 8], t)

                    # Wb transposes
                    for dc in range(2):
                        for ec in range(2):
                            t = ptr.tile([128, 128], f32, tag="tr", name="t")
                            nc.tensor.transpose(
                                t,
                                wbraw[:, dc * H + ec * 128: dc * H + (ec + 1) * 128],
                                ident)
                            pcopy(WbT[ec][:, dc * 128:(dc + 1) * 128], t)

                    # q: scale by scale_w/sqrt(DH) (DVE)
                    for mq in range(8):
                        qv = qraw[:, mq * H:(mq + 1) * H].rearrange(
                            "p (h j) -> p h j", h=NH)
                        nc.vector.scalar_tensor_tensor(
                            out=qv, in0=qv, scalar=ISQ,
                            in1=sc8[:, mq * 8:(mq + 1) * 8][:, :, None].broadcast_to(
                                [128, 8, 32]),
                            op0=Alu.mult, op1=Alu.mult)

                    def keff_mms(dc, pk):
                        for ec in range(2):
                            for ns in range(4):
                                nc.tensor.matmul(
                                    pk[:, ns * 512:(ns + 1) * 512],
                                    lhsT=WbT[ec][:, dc * 128:(dc + 1) * 128],
                                    rhs=kbT[ec][:, ns * 512:(ns + 1) * 512],
                                    start=(ec == 0), stop=False)
                        for ns in range(4):
                            nc.tensor.matmul(
                                pk[:, ns * 512:(ns + 1) * 512],
                                lhsT=bbb[0:1, dc * 128:(dc + 1) * 128],
                                rhs=oneslb[0:1, ns * 512:(ns + 1) * 512],
                                start=False, stop=True)
                        # evacuate with fused +k add; then 64-row-shifted copy
                        nc.vector.tensor_add(keffT[dc], pk, kT[dc])
                        nc.sync.dma_start(out=keffT2[dc][0:64],
                                          in_=keffT[dc][64:128])
                        nc.sync.dma_start(out=keffT2[dc][64:128],
                                          in_=keffT[dc][0:64])

                    pk0 = pkf.tile([128, L], f32, tag="pk", name="pk0")
                    keff_mms(0, pk0)

                    # q transposes into qsT
                    for dc in range(2):
                        for mq in range(8):
                            t = ptr.tile([128, 128], f32, tag="tr", name="t")
                            nc.tensor.transpose(
                                t,
                                qraw[:, mq * H + dc * 128: mq * H + (dc + 1) * 128],
                                ident)
                            pcopy(qsT[dc][:, mq * 128:(mq + 1) * 128], t)
                        nc.sync.dma_start(out=qsT2[dc][0:64],
                                          in_=qsT[dc][64:128])
                        nc.sync.dma_start(out=qsT2[dc][64:128],
                                          in_=qsT[dc][0:64])

                    pk1 = pkf.tile([128, L], f32, tag="pk", name="pk1")
                    keff_mms(1, pk1)

                    # Ww transposes (only needed at the end)
                    for er in range(2):
                        for g in range(2):
                            t = ptr.tile([128, 128], f32, tag="tr", name="t")
                            nc.tensor.transpose(
                                t,
                                wwraw[:, er * H + g * 128: er * H + (g + 1) * 128],
                                ident)
                            pcopy(WwT[g][:, er * 128:(er + 1) * 128], t)

            # ---------------- main attention loop ----------------
            # group g: heads (2g, 2g+1); chunk ch = g//2.
            # kc processed in pairs with alternating PE row groups (via the
            # 64-row-shifted tile copies): the pair's 4 QK matmuls occupy 4
            # distinct 32-row groups and stream concurrently.
            with (
                tc.tile_pool(name="pst", bufs=3, space="PSUM") as pst,
                tc.tile_pool(name="ppv", bufs=2, space="PSUM") as ppv,
            ):
                for g in range(4):
                    ch = g // 2
                    pv = [ppv.tile([128, 512], f32, tag="pv",
                                   name=f"pv{g}_{qb}") for qb in range(2)]
                    for kcp in range(8):
                        kcs = (2 * kcp, 2 * kcp + 1)
                        for qb in range(2):
                            sts2 = {}
                            for kc2 in kcs:
                                sts2[kc2] = pst.tile([128, 1024], f32,
                                                     tag="st", name=f"st{kc2 % 2}")
                            for kc2 in kcs:
                                par = kc2 % 2
                                kket = keffT[ch] if par == 0 else keffT2[ch]
                                qqt = qsT[ch] if par == 0 else qsT2[ch]
                                rbase = (g % 2) * 64 if par == 0 else (1 - g % 2) * 64
                                for t in range(2):
                                    ro = rbase + t * 32
                                    nc.tensor.matmul(
                                        sts2[kc2][:, t * 512:(t + 1) * 512],
                                        lhsT=kket[ro:ro + 32,
                                                  kc2 * 128:(kc2 + 1) * 128],
                                        rhs=qqt[ro:ro + 32,
                                                qb * 512:(qb + 1) * 512],
                                        tile_position=(ro, 0),
                                        start=True, stop=True)
                            # exp: qb0 -> ACT; qb1 -> DVE (Schraudolph),
                            # except every DVE_EXCL-th tile -> ACT (balance)
                            pts = {}
                            for kc2 in kcs:
                                io = g * 16 + kc2
                                if qb == 0 or io % DVE_EXCL == DVE_EXCL - 1:
                                    pt = ptp.tile([128, 1024], bf16, tag="ptA",
                                                  name="ptA")
                                    nc.scalar.activation(pt, sts2[kc2], Exp)
                                    pts[kc2] = pt
                                else:
                                    pti = ptp.tile([128, 1024], i16, tag="ptD",
                                                   name="ptD")
                                    nc.vector.tensor_scalar(
                                        out=pti, in0=sts2[kc2], scalar1=A16,
                                        scalar2=B16, op0=Alu.mult, op1=Alu.add)
                                    pts[kc2] = pti.bitcast(bf16)
                            # PV: h even lhsT=[m|v] -> rows [den|O];
                            #     h odd  lhsT=[v|m] -> rows [O|den]
                            for kc2 in kcs:
                                for t in range(2):
                                    h = 2 * g + t
                                    nc.tensor.matmul(
                                        pv[qb][64 * t:64 * t + 64, :],
                                        lhsT=vmm[:, (kc2 * NH + h) * 64:
                                                 (kc2 * NH + h) * 64 + 64],
                                        rhs=pts[kc2][:, t * 512:(t + 1) * 512],
                                        tile_position=(0, 64 * t),
                                        start=(kc2 == 0), stop=(kc2 == 15))
                    # normalize: pv rows = [den0 | O0 | O1 | den1].  Full
                    # 128-partition ops (custom DVE ops misbehave at nonzero
                    # partition base); unused lanes compute garbage, unread.
                    for qb in range(2):
                        ntmp = smp.tile([128, 512], f32, tag="ntmp", name="ntmp")
                        nc.vector.reciprocal_approx_fast(ntmp, pv[qb])
                        rtl = smp.tile([128, 512], f32, tag="rtl", name="rtl")
                        nc.sync.dma_start(out=rtl[32:64], in_=ntmp[0:32])
                        nc.sync.dma_start(out=rtl[64:96], in_=ntmp[96:128])
                        hst = smp.tile([128, 512], bf16, tag="hst", name="hst")
                        nc.vector.tensor_mul(hst, pv[qb], rtl)
                        ro2 = (g % 2) * 64
                        nc.sync.dma_start(
                            out=hidT[ch][ro2:ro2 + 64,
                                         qb * 512:(qb + 1) * 512],
                            in_=hst[32:96])

            # ---------------- output linear ----------------
            with tc.tile_pool(name="pout", bufs=2, space="PSUM") as pout:
                for mq in range(8):
                    po = pout.tile([128, H], f32, tag="po", name="po")
                    for gg in range(2):
                        nc.tensor.matmul(
                            po,
                            lhsT=hidT[gg][:, mq * 128:(mq + 1) * 128],
                            rhs=WwT[gg],
                            start=(gg == 0), stop=False)
                    nc.tensor.matmul(
                        po, lhsT=ones1b, rhs=bwb, start=False, stop=True)
                    if mq % 2 == 0:
                        nc.scalar.copy(outsb[:, mq * H:(mq + 1) * H], po)
                    else:
                        nc.vector.tensor_copy(outsb[:, mq * H:(mq + 1) * H], po)
                nc.sync.dma_start(
                    out=out_d.rearrange("(c p) e -> p c e", p=128),
                    in_=outsb.rearrange("p (c e) -> p c e", c=8))

    nc.compile()
    return nc


def _make_in_maps(inputs):
    q = np.ascontiguousarray(np.asarray(inputs["q"], dtype=np.float32))
    k = np.ascontiguousarray(np.asarray(inputs["k"], dtype=np.float32))
    v = np.ascontiguousarray(np.asarray(inputs["v"], dtype=np.float32))
    k_b = np.ascontiguousarray(np.asarray(inputs["k_b"], dtype=np.float32))
    mask = np.ascontiguousarray(np.asarray(inputs["mask"], dtype=np.int32))
    sw = np.ascontiguousarray(np.asarray(inputs["scale_w"], dtype=np.float32))
    Wb = np.ascontiguousarray(np.asarray(inputs["Wb"], dtype=np.float32))
    bb = np.ascontiguousarray(np.asarray(inputs["bb"], dtype=np.float32))
    Ww = np.ascontiguousarray(np.asarray(inputs["Ww"], dtype=np.float32))
    bw = np.ascontiguousarray(np.asarray(inputs["bw"], dtype=np.float32))
    ident = np.eye(128, dtype=np.float32)
    in_maps = []
    for c in range(NCORES):
        b, qs = c // 2, c % 2
        in_maps.append({
            "q_s": q[b, qs * LQ:(qs + 1) * LQ, :],
            "k_s": k[b],
            "v_s": v[b],
            "kb_s": k_b[b],
            "mask_s": mask[b],
            "sw_s": np.ascontiguousarray(sw[:, qs * LQ:(qs + 1) * LQ]),
            "Wb": Wb, "bb": bb, "Ww": Ww, "bw": bw,
            "ident": ident,
        })
    return in_maps


def _enable_ldw_opt():
    """Rewrite walrus's hardcoded --enable-ldw-opt=false: identical
    back-to-back weight loads are elided, keeping the PE stream dense."""
    from concourse import bass_utils as bu

    if getattr(bu, "_ldw_patched", False):
        return
    orig = bu.run_command

    def patched(argv, **kwargs):
        # --enable-ldw-opt=true breaks codegen for tile-positioned
        # LDWEIGHTS ("InstLdweights is not compatible with LDW
        # optimization"); leave the flag alone.
        return orig(argv, **kwargs)

    bu.run_command = patched
    bu._ldw_patched = True


def run_sharded(inputs, trace=False, tmpdir=None):
    from concourse import bass_utils
    from concourse.bass_utils import run_bass_kernel_spmd

    _enable_ldw_opt()
    if trace:
        _install_ntff_hook()
        bass_utils.upload_artifacts = lambda d: d
    nc = _build()
    in_maps = _make_in_maps(inputs)
    res = run_bass_kernel_spmd(nc, in_maps, list(range(NCORES)),
                               trace=trace, tmpdir=tmpdir)
    out = np.empty((B, L, H), dtype=np.float32)
    for c in range(NCORES):
        b, qs = c // 2, c % 2
        out[b, qs * LQ:(qs + 1) * LQ, :] = res.results[c]["out"]
    return out, res


def kernel(**inputs):
    out, _ = run_sharded(inputs, trace=False)
    return out


def _install_ntff_hook():
    """Provide antenv.axon_hooks (absent in this image) so trace=True works."""
    import contextlib
    import ctypes
    import types

    import antenv

    if hasattr(antenv, "axon_hooks"):
        return
    mod = types.ModuleType("antenv.axon_hooks")
    _hook = [None]
    mod.set_axon_ntff_profile_hook = lambda h: _hook.__setitem__(0, h)
    mod.get_axon_ntff_profile_hook = lambda: _hook[0]
    antenv.axon_hooks = mod
    sys.modules["antenv.axon_hooks"] = mod

    lib = ctypes.CDLL("/opt/axon/libaxon_pjrt.so")
    if not hasattr(lib, "axon_start_nrt_profile"):
        return
    lib.axon_start_nrt_profile.argtypes = [ctypes.POINTER(ctypes.c_int64),
                                           ctypes.c_size_t]
    lib.axon_start_nrt_profile.restype = ctypes.c_int64
    lib.axon_stop_nrt_profile.argtypes = [ctypes.c_char_p]
    lib.axon_stop_nrt_profile.restype = ctypes.c_int64

    @contextlib.contextmanager
    def _profile(output_dir, device_ids):
        import jax

        jax.devices()
        if device_ids:
            ids = (ctypes.c_int64 * len(device_ids))(*device_ids)
            rc = lib.axon_start_nrt_profile(ids, len(device_ids))
        else:
            rc = lib.axon_start_nrt_profile(None, 0)
        if rc != 0:
            raise RuntimeError(f"axon_start_nrt_profile rc={rc}")
        try:
            yield
        finally:
            n = lib.axon_stop_nrt_profile(str(output_dir).encode())
            print(f"profile: {n} file(s) written to {output_dir}",
                  file=sys.stderr)

    mod.set_axon_ntff_profile_hook(_profile)


# revision 10
# speedup vs baseline: 1.0119x; 1.0119x over previous
"""Trainium2 Bass kernel for AuxiliaryMultiHeadedAttention.

Reference computation (B=4, L=2048, H=256, NH=8, DH=32):
    kb   = split_heads(k_b @ Wb.T + bb)
    corr = (qh @ kh^T + qh @ kb^T) / sqrt(DH) * scale_w[h, q]
    corr = where(mask==0, -1e9, corr);  prob = softmax(corr)
    out  = merge_heads(prob @ vh) @ Ww.T + bw

Kernel strategy (8 NeuronCores):
    Shard (batch, query-half): core c -> batch c//2, queries (c%2)*1024..+1024.
    Each core:
      kT, kbT via bf16 cast + DMA-xbar transpose (no PE transposes)
      keffT = (k + k_b @ Wb.T + bb)^T  [dims, keys]  bf16 (dual QK^T folded)
      qsT   = (q * scale_w/sqrt(DH))^T [dims, queries] bf16
      S^T   = keffT_h^T @ qsT_h  (bf16 MMs, 2 heads row-tiled, fp32 psum;
              row groups alternate with kc parity via 64-row-shifted tile
              copies so LDWEIGHTS overlaps the previous chunk's matmuls)
      P^T   = exp(S^T): split between ACT (exact exp, bf16 out) and DVE
              (Schraudolph: int16(A*x+B) bitcast to bf16, one tensor_scalar)
      PV with lhsT [m|v_h] / [v_h|m] (m = mask: masks both numerator and
              denominator) -> psum rows [den0|O0|O1|den1]
      hidT  = O * recip(den)  (full-partition recip + mul, DMA realign)
      out   = hidT^T @ WwT + bw  (bf16 MMs)
    Host concatenates the 8 [1024, 256] slices.
"""

import sys

if "/opt/trn_rl_repo" not in sys.path:
    sys.path.insert(0, "/opt/trn_rl_repo")

import math

import numpy as np

B, L, H, NH, DH = 4, 2048, 256, 8, 32
LQ = 1024  # queries per core
NCORES = 8
ISQ = 1.0 / math.sqrt(DH)

# Schraudolph exp for bf16 target: bf16bits(exp(x)) ~ int16(A16*x + B16)
A16 = 128.0 / math.log(2.0)
C_OFF = 5.5
B16 = 127.0 * 128.0 - C_OFF
# every DVE_EXCL-th qb1 exp tile goes to ACT instead of DVE (load balance)
DVE_EXCL = 4


def _build():
    import concourse.bass as bass  # noqa: F401
    import concourse.mybir as mybir
    import concourse.tile as tile
    from concourse import bacc

    f32 = mybir.dt.float32
    i32 = mybir.dt.int32
    i16 = mybir.dt.int16
    bf16 = mybir.dt.bfloat16
    Exp = mybir.ActivationFunctionType.Exp
    Alu = mybir.AluOpType

    nc = bacc.Bacc("TRN2", target_bir_lowering=False, debug=False, num_devices=NCORES)

    q_d = nc.dram_tensor("q_s", [LQ, H], f32, kind="ExternalInput")
    k_d = nc.dram_tensor("k_s", [L, H], f32, kind="ExternalInput")
    v_d = nc.dram_tensor("v_s", [L, H], f32, kind="ExternalInput")
    kb_d = nc.dram_tensor("kb_s", [L, H], f32, kind="ExternalInput")
    mask_d = nc.dram_tensor("mask_s", [L], i32, kind="ExternalInput")
    sw_d = nc.dram_tensor("sw_s", [NH, LQ], f32, kind="ExternalInput")
    Wb_d = nc.dram_tensor("Wb", [H, H], f32, kind="ExternalInput")
    bb_d = nc.dram_tensor("bb", [H], f32, kind="ExternalInput")
    Ww_d = nc.dram_tensor("Ww", [H, H], f32, kind="ExternalInput")
    bw_d = nc.dram_tensor("bw", [H], f32, kind="ExternalInput")
    id_d = nc.dram_tensor("ident", [128, 128], f32, kind="ExternalInput")
    out_d = nc.dram_tensor("out", [LQ, H], f32, kind="ExternalOutput")

    copy_flip = [0]

    with tile.TileContext(nc) as tc:
        with (
            tc.tile_pool(name="persist", bufs=1) as pp,
            tc.tile_pool(name="pt", bufs=3) as ptp,
            tc.tile_pool(name="small", bufs=2) as smp,
        ):
            # ---------------- persistent SBUF tensors ----------------
            ident = pp.tile([128, 128], f32, tag="ident")
            nc.sync.dma_start(out=ident, in_=id_d[:, :])
            keffT = [pp.tile([128, L], bf16, tag=f"keffT{g}", name=f"keffT{g}")
                     for g in range(2)]
            keffT2 = [pp.tile([128, L], bf16, tag=f"keffT2_{g}",
                              name=f"keffT2_{g}") for g in range(2)]
            qsT = [pp.tile([128, LQ], bf16, tag=f"qsT{g}", name=f"qsT{g}")
                   for g in range(2)]
            qsT2 = [pp.tile([128, LQ], bf16, tag=f"qsT2_{g}", name=f"qsT2_{g}")
                    for g in range(2)]
            # per (key-chunk, head): [m|v_h] (h even) / [v_h|m] (h odd);
            # m = mask column (masks numerator and denominator)
            vmm = pp.tile([128, 16 * NH * 64], bf16, tag="vmm")
            hidT = [pp.tile([128, LQ], bf16, tag=f"hidT{g}", name=f"hidT{g}")
                    for g in range(2)]
            WwT = [pp.tile([128, H], bf16, tag=f"WwT{g}", name=f"WwT{g}")
                   for g in range(2)]
            bwrep = pp.tile([128, H], f32, tag="bwrep")
            sc8 = pp.tile([128, 64], f32, tag="sc8")
            outsb = pp.tile([128, 8 * H], f32, tag="outsb")

            with tc.tile_pool(name="stage", bufs=1) as sp:
                def pcopy(dst, src):
                    # alternate psum->sbuf evacuation between DVE and ACT
                    if copy_flip[0] % 2 == 0:
                        nc.vector.tensor_copy(dst, src)
                    else:
                        nc.scalar.copy(dst, src)
                    copy_flip[0] += 1

                # ---------------- staging loads ----------------
                # spread across the DMA issue queues: sync (q/v/weights),
                # scalar (k_b chunks + kT transposes), gpsimd (k chunks)
                swt = sp.tile([NH, LQ], f32, tag="swt")
                nc.sync.dma_start(out=swt, in_=sw_d[:, :])
                qraw = sp.tile([128, 8 * H], f32, tag="qraw")
                nc.sync.dma_start(out=qraw.rearrange("p (c e) -> p c e", c=8),
                                  in_=q_d.rearrange("(c p) e -> p c e", p=128))
                wbraw = sp.tile([128, 2 * H], f32, tag="wbraw")
                nc.sync.dma_start(out=wbraw.rearrange("p (c e) -> p c e", c=2),
                                  in_=Wb_d.rearrange("(c p) e -> p c e", p=128))
                kbf = [sp.tile([128, 16 * 128], f32, tag=f"kbf{e}",
                               name=f"kbf{e}") for e in range(2)]
                kf = [sp.tile([128, 16 * 128], f32, tag=f"kf{e}",
                              name=f"kf{e}") for e in range(2)]
                kb16 = [sp.tile([128, 16 * 128], bf16, tag=f"kb16_{e}",
                                name=f"kb16_{e}") for e in range(2)]
                k16 = [sp.tile([128, 16 * 128], bf16, tag=f"k16_{e}",
                               name=f"k16_{e}") for e in range(2)]
                kbT = [sp.tile([128, L], bf16, tag=f"kbT{e}", name=f"kbT{e}")
                       for e in range(2)]
                kT = [sp.tile([128, L], bf16, tag=f"kT{e}", name=f"kT{e}")
                      for e in range(2)]
                dvkb = kb_d.rearrange("(c p) e -> p c e", p=128)
                dvk = k_d.rearrange("(c p) e -> p c e", p=128)
                # chunked load -> cast -> xbar-transpose pipeline
                for ec in range(2):
                    tvb = kbf[ec].rearrange("p (c f) -> p c f", c=16)
                    tvk = kf[ec].rearrange("p (c f) -> p c f", c=16)
                    for cq in range(2):
                        cs = slice(cq * 8, (cq + 1) * 8)
                        nc.scalar.dma_start(
                            out=tvb[:, cs, :],
                            in_=dvkb[:, cs, ec * 128:(ec + 1) * 128])
                        nc.gpsimd.dma_start(
                            out=tvk[:, cs, :],
                            in_=dvk[:, cs, ec * 128:(ec + 1) * 128])
                        co = slice(cq * 1024, (cq + 1) * 1024)
                        nc.vector.tensor_copy(kb16[ec][:, co], kbf[ec][:, co])
                        nc.vector.tensor_copy(k16[ec][:, co], kf[ec][:, co])
                        nc.sync.dma_start_transpose(
                            out=kbT[ec].rearrange("p (c f) -> p c f",
                                                  c=16)[:, cq * 8:(cq + 1) * 8, :],
                            in_=kb16[ec][:, co])
                        nc.scalar.dma_start_transpose(
                            out=kT[ec].rearrange("p (c f) -> p c f",
                                                 c=16)[:, cq * 8:(cq + 1) * 8, :],
                            in_=k16[ec][:, co])

                vraw = sp.tile([128, 16 * H], f32, tag="vraw")
                vv = vraw.rearrange("p (c e) -> p c e", c=16)
                dvv = v_d.rearrange("(c p) e -> p c e", p=128)
                for c4 in range(2):
                    nc.sync.dma_start(out=vv[:, c4 * 8:(c4 + 1) * 8, :],
                                      in_=dvv[:, c4 * 8:(c4 + 1) * 8, :])
                m16 = sp.tile([16, 128], i32, tag="m16")
                nc.sync.dma_start(out=m16,
                                  in_=mask_d.rearrange("(c p) -> c p", p=128))
                wwraw = sp.tile([128, 2 * H], f32, tag="wwraw")
                nc.sync.dma_start(out=wwraw.rearrange("p (c e) -> p c e", c=2),
                                  in_=Ww_d.rearrange("(c p) e -> p c e", p=128))
                bbt = sp.tile([1, H], f32, tag="bbt")
                nc.sync.dma_start(out=bbt, in_=bb_d[None, :])
                bbb = sp.tile([1, H], bf16, tag="bbb")
                nc.vector.tensor_copy(bbb, bbt)
                bwt = sp.tile([1, H], f32, tag="bwt")
                nc.sync.dma_start(out=bwt, in_=bw_d[None, :])
                # bwrep: bw broadcast to 128 partitions via doubling DMAs
                nc.gpsimd.dma_start(out=bwrep[0:1], in_=bwt)
                for db in range(7):
                    n = 1 << db
                    nc.gpsimd.dma_start(out=bwrep[n:2 * n], in_=bwrep[0:n])
                oneslb = sp.tile([1, L], bf16, tag="oneslb")
                nc.vector.memset(oneslb, 1.0)
                m16f = sp.tile([16, 128], f32, tag="m16f")
                nc.vector.tensor_copy(m16f, m16)
                maskf = sp.tile([128, 16], f32, tag="maskf")
                WbT = [sp.tile([128, H], bf16, tag=f"WbT{e}", name=f"WbT{e}")
                       for e in range(2)]

                # ---------------- prep: transposes & keff ----------------
                with (
                    tc.tile_pool(name="ptr", bufs=4, space="PSUM") as ptr,
                    tc.tile_pool(name="pkeff", bufs=1, space="PSUM") as pkf,
                ):
                    # mask -> maskf [128, 16]
                    tm = ptr.tile([128, 16], f32, tag="tr")
                    nc.tensor.transpose(tm, m16f, ident[0:16, 0:16])
                    nc.vector.tensor_copy(maskf, tm)

                    # vmm: [m|v]/[v|m] layout; v and m both mask-scaled.
                    # v-cols on DVE, m-cols on ACT; split by key-chunk half
                    # so the second vraw load chunk overlaps the first's build
                    vmm6 = vmm.rearrange("p (c hp par two d) -> p c hp par two d",
                                         c=16, hp=4, par=2, two=2)
                    vraw4 = vraw.rearrange("p (c hp par d) -> p c hp par d",
                                           c=16, hp=4, par=2)
                    for chalf in range(2):
                        cs = slice(chalf * 8, (chalf + 1) * 8)
                        for par in range(2):
                            for hp in range(4):
                                nc.vector.scalar_tensor_tensor(
                                    out=vmm6[:, cs, hp, par, 1 - par, :],
                                    in0=vraw4[:, cs, hp, par, :], scalar=1.0,
                                    in1=maskf[:, cs, None].broadcast_to(
                                        [128, 8, 32]),
                                    op0=Alu.mult, op1=Alu.mult)
                                nc.scalar.copy(
                                    vmm6[:, cs, hp, par, par, :],
                                    maskf[:, cs, None].broadcast_to(
                                        [128, 8, 32]))

                    # scale_w slices -> sc8 [128, 8 per q-chunk]
                    for mq in range(8):
                        t = ptr.tile([128, 8], f32, tag="tr", name="t")
                        nc.tensor.transpose(t, swt[:, mq * 128:(mq + 1) * 128],
                                            ident[0:NH, 0:NH])
                        nc.vector.tensor_copy(sc8[:, mq * 8:(mq + 1) * 8], t)

                    # Wb transposes
                    for dc in range(2):
                        for ec in range(2):
                            t = ptr.tile([128, 128], f32, tag="tr", name="t")
                            nc.tensor.transpose(
                                t,
                                wbraw[:, dc * H + ec * 128: dc * H + (ec + 1) * 128],
                                ident)
                            pcopy(WbT[ec][:, dc * 128:(dc + 1) * 128], t)

                    # q: scale by scale_w/sqrt(DH) (DVE)
                    for mq in range(8):
                        qv = qraw[:, mq * H:(mq + 1) * H].rearrange(
                            "p (h j) -> p h j", h=NH)
                        nc.vector.scalar_tensor_tensor(
                            out=qv, in0=qv, scalar=ISQ,
                            in1=sc8[:, mq * 8:(mq + 1) * 8][:, :, None].broadcast_to(
                                [128, 8, 32]),
                            op0=Alu.mult, op1=Alu.mult)

                    def keff_mms(dc, pk):
                        for ec in range(2):
                            for ns in range(4):
                                nc.tensor.matmul(
                                    pk[:, ns * 512:(ns + 1) * 512],
                                    lhsT=WbT[ec][:, dc * 128:(dc + 1) * 128],
                                    rhs=kbT[ec][:, ns * 512:(ns + 1) * 512],
                                    start=(ec == 0), stop=False)
                        for ns in range(4):
                            nc.tensor.matmul(
                                pk[:, ns * 512:(ns + 1) * 512],
                                lhsT=bbb[0:1, dc * 128:(dc + 1) * 128],
                                rhs=oneslb[0:1, ns * 512:(ns + 1) * 512],
                                start=False, stop=True)
                        # evacuate with fused +k add; then 64-row-shifted copy
                        for nh2 in range(2):
                            co = slice(nh2 * 1024, (nh2 + 1) * 1024)
                            nc.vector.tensor_add(keffT[dc][:, co], pk[:, co],
                                                 kT[dc][:, co])
                        nc.sync.dma_start(out=keffT2[dc][0:64],
                                          in_=keffT[dc][64:128])
                        nc.sync.dma_start(out=keffT2[dc][64:128],
                                          in_=keffT[dc][0:64])

                    pk0 = pkf.tile([128, L], f32, tag="pk", name="pk0")
                    keff_mms(0, pk0)

                    # q transposes into qsT
                    for dc in range(2):
                        for mq in range(8):
                            t = ptr.tile([128, 128], f32, tag="tr", name="t")
                            nc.tensor.transpose(
                                t,
                                qraw[:, mq * H + dc * 128: mq * H + (dc + 1) * 128],
                                ident)
                            pcopy(qsT[dc][:, mq * 128:(mq + 1) * 128], t)
                        nc.sync.dma_start(out=qsT2[dc][0:64],
                                          in_=qsT[dc][64:128])
                        nc.sync.dma_start(out=qsT2[dc][64:128],
                                          in_=qsT[dc][0:64])

                    pk1 = pkf.tile([128, L], f32, tag="pk", name="pk1")
                    keff_mms(1, pk1)

                    # Ww transposes (only needed at the end)
                    for er in range(2):
                        for g in range(2):
                            t = ptr.tile([128, 128], f32, tag="tr", name="t")
                            nc.tensor.transpose(
                                t,
                                wwraw[:, er * H + g * 128: er * H + (g + 1) * 128],
                                ident)
                            pcopy(WwT[g][:, er * 128:(er + 1) * 128], t)

            # ---------------- main attention loop ----------------
            # group g: heads (2g, 2g+1); chunk ch = g//2.
            # kc processed in pairs with alternating PE row groups (via the
            # 64-row-shifted tile copies): the pair's 4 QK matmuls occupy 4
            # distinct 32-row groups and stream concurrently.
            with (
                tc.tile_pool(name="pst", bufs=3, space="PSUM") as pst,
                tc.tile_pool(name="ppv", bufs=2, space="PSUM") as ppv,
            ):
                for g in range(4):
                    ch = g // 2
                    pv = [ppv.tile([128, 512], f32, tag="pv",
                                   name=f"pv{g}_{qb}") for qb in range(2)]
                    for kcp in range(8):
                        kcs = (2 * kcp, 2 * kcp + 1)
                        for qb in range(2):
                            sts2 = {}
                            for kc2 in kcs:
                                sts2[kc2] = pst.tile([128, 1024], f32,
                                                     tag="st", name=f"st{kc2 % 2}")
                            for kc2 in kcs:
                                par = kc2 % 2
                                kket = keffT[ch] if par == 0 else keffT2[ch]
                                qqt = qsT[ch] if par == 0 else qsT2[ch]
                                rbase = (g % 2) * 64 if par == 0 else (1 - g % 2) * 64
                                for t in range(2):
                                    ro = rbase + t * 32
                                    nc.tensor.matmul(
                                        sts2[kc2][:, t * 512:(t + 1) * 512],
                                        lhsT=kket[ro:ro + 32,
                                                  kc2 * 128:(kc2 + 1) * 128],
                                        rhs=qqt[ro:ro + 32,
                                                qb * 512:(qb + 1) * 512],
                                        tile_position=(ro, 0),
                                        start=True, stop=True)
                            # exp: qb0 -> ACT; qb1 -> DVE (Schraudolph),
                            # except every DVE_EXCL-th tile -> ACT (balance)
                            pts = {}
                            for kc2 in kcs:
                                io = g * 16 + kc2
                                if qb == 0 or io % DVE_EXCL == DVE_EXCL - 1:
                                    pt = ptp.tile([128, 1024], bf16, tag="ptA",
                                                  name="ptA")
                                    nc.scalar.activation(pt, sts2[kc2], Exp)
                                    pts[kc2] = pt
                                else:
                                    pti = ptp.tile([128, 1024], i16, tag="ptD",
                                                   name="ptD")
                                    nc.vector.tensor_scalar(
                                        out=pti, in0=sts2[kc2], scalar1=A16,
                                        scalar2=B16, op0=Alu.mult, op1=Alu.add)
                                    pts[kc2] = pti.bitcast(bf16)
                            # PV: h even lhsT=[m|v] -> rows [den|O];
                            #     h odd  lhsT=[v|m] -> rows [O|den]
                            for kc2 in kcs:
                                for t in range(2):
                                    h = 2 * g + t
                                    nc.tensor.matmul(
                                        pv[qb][64 * t:64 * t + 64, :],
                                        lhsT=vmm[:, (kc2 * NH + h) * 64:
                                                 (kc2 * NH + h) * 64 + 64],
                                        rhs=pts[kc2][:, t * 512:(t + 1) * 512],
                                        tile_position=(0, 64 * t),
                                        start=(kc2 == 0), stop=(kc2 == 15))
                    # normalize: pv rows = [den0 | O0 | O1 | den1].  Full
                    # 128-partition ops (custom DVE ops misbehave at nonzero
                    # partition base); unused lanes compute garbage, unread.
                    for qb in range(2):
                        ntmp = smp.tile([128, 512], f32, tag="ntmp", name="ntmp")
                        nc.vector.reciprocal_approx_fast(ntmp, pv[qb])
                        rtl = smp.tile([128, 512], f32, tag="rtl", name="rtl")
                        nc.sync.dma_start(out=rtl[32:64], in_=ntmp[0:32])
                        nc.sync.dma_start(out=rtl[64:96], in_=ntmp[96:128])
                        hst = smp.tile([128, 512], bf16, tag="hst", name="hst")
                        nc.vector.tensor_mul(hst, pv[qb], rtl)
                        ro2 = (g % 2) * 64
                        nc.sync.dma_start(
                            out=hidT[ch][ro2:ro2 + 64,
                                         qb * 512:(qb + 1) * 512],
                            in_=hst[32:96])

            # ---------------- output linear ----------------
            with tc.tile_pool(name="pout", bufs=2, space="PSUM") as pout:
                for mq in range(8):
                    po = pout.tile([128, H], f32, tag="po", name="po")
                    for gg in range(2):
                        nc.tensor.matmul(
                            po,
                            lhsT=hidT[gg][:, mq * 128:(mq + 1) * 128],
                            rhs=WwT[gg],
                            start=(gg == 0), stop=(gg == 1))
                    nc.vector.tensor_add(outsb[:, mq * H:(mq + 1) * H],
                                          po, bwrep)
                nc.sync.dma_start(
                    out=out_d.rearrange("(c p) e -> p c e", p=128),
                    in_=outsb.rearrange("p (c e) -> p c e", c=8))

    nc.compile()
    return nc


def _make_in_maps(inputs):
    q = np.ascontiguousarray(np.asarray(inputs["q"], dtype=np.float32))
    k = np.ascontiguousarray(np.asarray(inputs["k"], dtype=np.float32))
    v = np.ascontiguousarray(np.asarray(inputs["v"], dtype=np.float32))
    k_b = np.ascontiguousarray(np.asarray(inputs["k_b"], dtype=np.float32))
    mask = np.ascontiguousarray(np.asarray(inputs["mask"], dtype=np.int32))
    sw = np.ascontiguousarray(np.asarray(inputs["scale_w"], dtype=np.float32))
    Wb = np.ascontiguousarray(np.asarray(inputs["Wb"], dtype=np.float32))
    bb = np.ascontiguousarray(np.asarray(inputs["bb"], dtype=np.float32))
    Ww = np.ascontiguousarray(np.asarray(inputs["Ww"], dtype=np.float32))
    bw = np.ascontiguousarray(np.asarray(inputs["bw"], dtype=np.float32))
    ident = np.eye(128, dtype=np.float32)
    in_maps = []
    for c in range(NCORES):
        b, qs = c // 2, c % 2
        in_maps.append({
            "q_s": q[b, qs * LQ:(qs + 1) * LQ, :],
            "k_s": k[b],
            "v_s": v[b],
            "kb_s": k_b[b],
            "mask_s": mask[b],
            "sw_s": np.ascontiguousarray(sw[:, qs * LQ:(qs + 1) * LQ]),
            "Wb": Wb, "bb": bb, "Ww": Ww, "bw": bw,
            "ident": ident,
        })
    return in_maps


def _enable_ldw_opt():
    """Rewrite walrus's hardcoded --enable-ldw-opt=false: identical
    back-to-back weight loads are elided, keeping the PE stream dense."""
    from concourse import bass_utils as bu

    if getattr(bu, "_ldw_patched", False):
        return
    orig = bu.run_command

    def patched(argv, **kwargs):
        # --enable-ldw-opt=true breaks codegen for tile-positioned
        # LDWEIGHTS ("InstLdweights is not compatible with LDW
        # optimization"); leave the flag alone.
        return orig(argv, **kwargs)

    bu.run_command = patched
    bu._ldw_patched = True


def run_sharded(inputs, trace=False, tmpdir=None):
    from concourse import bass_utils
    from concourse.bass_utils import run_bass_kernel_spmd

    _enable_ldw_opt()
    if trace:
        _install_ntff_hook()
        bass_utils.upload_artifacts = lambda d: d
    nc = _build()
    in_maps = _make_in_maps(inputs)
    res = run_bass_kernel_spmd(nc, in_maps, list(range(NCORES)),
                               trace=trace, tmpdir=tmpdir)
    out = np.empty((B, L, H), dtype=np.float32)
    for c in range(NCORES):
        b, qs = c // 2, c % 2
        out[b, qs * LQ:(qs + 1) * LQ, :] = res.results[c]["out"]
    return out, res


def kernel(**inputs):
    out, _ = run_sharded(inputs, trace=False)
    return out


def _install_ntff_hook():
    """Provide antenv.axon_hooks (absent in this image) so trace=True works."""
    import contextlib
    import ctypes
    import types

    import antenv

    if hasattr(antenv, "axon_hooks"):
        return
    mod = types.ModuleType("antenv.axon_hooks")
    _hook = [None]
    mod.set_axon_ntff_profile_hook = lambda h: _hook.__setitem__(0, h)
    mod.get_axon_ntff_profile_hook = lambda: _hook[0]
    antenv.axon_hooks = mod
    sys.modules["antenv.axon_hooks"] = mod

    lib = ctypes.CDLL("/opt/axon/libaxon_pjrt.so")
    if not hasattr(lib, "axon_start_nrt_profile"):
        return
    lib.axon_start_nrt_profile.argtypes = [ctypes.POINTER(ctypes.c_int64),
                                           ctypes.c_size_t]
    lib.axon_start_nrt_profile.restype = ctypes.c_int64
    lib.axon_stop_nrt_profile.argtypes = [ctypes.c_char_p]
    lib.axon_stop_nrt_profile.restype = ctypes.c_int64

    @contextlib.contextmanager
    def _profile(output_dir, device_ids):
        import jax

        jax.devices()
        if device_ids:
            ids = (ctypes.c_int64 * len(device_ids))(*device_ids)
            rc = lib.axon_start_nrt_profile(ids, len(device_ids))
        else:
            rc = lib.axon_start_nrt_profile(None, 0)
        if rc != 0:
            raise RuntimeError(f"axon_start_nrt_profile rc={rc}")
        try:
            yield
        finally:
            n = lib.axon_stop_nrt_profile(str(output_dir).encode())
            print(f"profile: {n} file(s) written to {output_dir}",
                  file=sys.stderr)

    mod.set_axon_ntff_profile_hook(_profile)


# revision 11
# speedup vs baseline: 1.0169x; 1.0050x over previous
"""Trainium2 Bass kernel for AuxiliaryMultiHeadedAttention.

Reference computation (B=4, L=2048, H=256, NH=8, DH=32):
    kb   = split_heads(k_b @ Wb.T + bb)
    corr = (qh @ kh^T + qh @ kb^T) / sqrt(DH) * scale_w[h, q]
    corr = where(mask==0, -1e9, corr);  prob = softmax(corr)
    out  = merge_heads(prob @ vh) @ Ww.T + bw

Kernel strategy (8 NeuronCores):
    Shard (batch, query-half): core c -> batch c//2, queries (c%2)*1024..+1024.
    Each core:
      kT, kbT via bf16 cast + DMA-xbar transpose (no PE transposes)
      keffT = (k + k_b @ Wb.T + bb)^T  [dims, keys]  bf16 (dual QK^T folded)
      qsT   = (q * scale_w/sqrt(DH))^T [dims, queries] bf16
      S^T   = keffT_h^T @ qsT_h  (bf16 MMs, 2 heads row-tiled, fp32 psum;
              row groups alternate with kc parity via 64-row-shifted tile
              copies so LDWEIGHTS overlaps the previous chunk's matmuls)
      P^T   = exp(S^T): split between ACT (exact exp, bf16 out) and DVE
              (Schraudolph: int16(A*x+B) bitcast to bf16, one tensor_scalar)
      PV with lhsT [m|v_h] / [v_h|m] (m = mask: masks both numerator and
              denominator) -> psum rows [den0|O0|O1|den1]
      hidT  = O * recip(den)  (full-partition recip + mul, DMA realign)
      out   = hidT^T @ WwT + bw  (bf16 MMs)
    Host concatenates the 8 [1024, 256] slices.
"""

import sys

if "/opt/trn_rl_repo" not in sys.path:
    sys.path.insert(0, "/opt/trn_rl_repo")

import math

import numpy as np

B, L, H, NH, DH = 4, 2048, 256, 8, 32
LQ = 1024  # queries per core
NCORES = 8
ISQ = 1.0 / math.sqrt(DH)

# Schraudolph exp for bf16 target: bf16bits(exp(x)) ~ int16(A16*x + B16)
A16 = 128.0 / math.log(2.0)
C_OFF = 5.5
B16 = 127.0 * 128.0 - C_OFF
# every DVE_EXCL-th qb1 exp tile goes to ACT instead of DVE (load balance)
DVE_EXCL = 6


def _build():
    import concourse.bass as bass  # noqa: F401
    import concourse.mybir as mybir
    import concourse.tile as tile
    from concourse import bacc

    f32 = mybir.dt.float32
    i32 = mybir.dt.int32
    i16 = mybir.dt.int16
    bf16 = mybir.dt.bfloat16
    Exp = mybir.ActivationFunctionType.Exp
    Alu = mybir.AluOpType

    nc = bacc.Bacc("TRN2", target_bir_lowering=False, debug=False, num_devices=NCORES)

    q_d = nc.dram_tensor("q_s", [LQ, H], f32, kind="ExternalInput")
    k_d = nc.dram_tensor("k_s", [L, H], f32, kind="ExternalInput")
    v_d = nc.dram_tensor("v_s", [L, H], f32, kind="ExternalInput")
    kb_d = nc.dram_tensor("kb_s", [L, H], f32, kind="ExternalInput")
    mask_d = nc.dram_tensor("mask_s", [L], i32, kind="ExternalInput")
    sw_d = nc.dram_tensor("sw_s", [NH, LQ], f32, kind="ExternalInput")
    Wb_d = nc.dram_tensor("Wb", [H, H], f32, kind="ExternalInput")
    bb_d = nc.dram_tensor("bb", [H], f32, kind="ExternalInput")
    Ww_d = nc.dram_tensor("Ww", [H, H], f32, kind="ExternalInput")
    bw_d = nc.dram_tensor("bw", [H], f32, kind="ExternalInput")
    id_d = nc.dram_tensor("ident", [128, 128], f32, kind="ExternalInput")
    out_d = nc.dram_tensor("out", [LQ, H], f32, kind="ExternalOutput")

    copy_flip = [0]

    with tile.TileContext(nc) as tc:
        with (
            tc.tile_pool(name="persist", bufs=1) as pp,
            tc.tile_pool(name="pt", bufs=3) as ptp,
            tc.tile_pool(name="small", bufs=2) as smp,
        ):
            # ---------------- persistent SBUF tensors ----------------
            ident = pp.tile([128, 128], f32, tag="ident")
            nc.sync.dma_start(out=ident, in_=id_d[:, :])
            keffT = [pp.tile([128, L], bf16, tag=f"keffT{g}", name=f"keffT{g}")
                     for g in range(2)]
            keffT2 = [pp.tile([128, L], bf16, tag=f"keffT2_{g}",
                              name=f"keffT2_{g}") for g in range(2)]
            qsT = [pp.tile([128, LQ], bf16, tag=f"qsT{g}", name=f"qsT{g}")
                   for g in range(2)]
            qsT2 = [pp.tile([128, LQ], bf16, tag=f"qsT2_{g}", name=f"qsT2_{g}")
                    for g in range(2)]
            # per (key-chunk, head): [m|v_h] (h even) / [v_h|m] (h odd);
            # m = mask column (masks numerator and denominator)
            vmm = pp.tile([128, 16 * NH * 64], bf16, tag="vmm")
            hidT = [pp.tile([128, LQ], bf16, tag=f"hidT{g}", name=f"hidT{g}")
                    for g in range(2)]
            WwT = [pp.tile([128, H], bf16, tag=f"WwT{g}", name=f"WwT{g}")
                   for g in range(2)]
            bwrep = pp.tile([128, H], f32, tag="bwrep")
            sc8 = pp.tile([128, 64], f32, tag="sc8")
            outsb = pp.tile([128, 8 * H], f32, tag="outsb")

            with tc.tile_pool(name="stage", bufs=1) as sp:
                def pcopy(dst, src):
                    # alternate psum->sbuf evacuation between DVE and ACT
                    if copy_flip[0] % 2 == 0:
                        nc.vector.tensor_copy(dst, src)
                    else:
                        nc.scalar.copy(dst, src)
                    copy_flip[0] += 1

                # ---------------- staging loads ----------------
                # spread across the DMA issue queues: sync (q/v/weights),
                # scalar (k_b chunks + kT transposes), gpsimd (k chunks)
                swt = sp.tile([NH, LQ], f32, tag="swt")
                nc.sync.dma_start(out=swt, in_=sw_d[:, :])
                qraw = sp.tile([128, 8 * H], f32, tag="qraw")
                nc.sync.dma_start(out=qraw.rearrange("p (c e) -> p c e", c=8),
                                  in_=q_d.rearrange("(c p) e -> p c e", p=128))
                wbraw = sp.tile([128, 2 * H], f32, tag="wbraw")
                nc.sync.dma_start(out=wbraw.rearrange("p (c e) -> p c e", c=2),
                                  in_=Wb_d.rearrange("(c p) e -> p c e", p=128))
                kbf = [sp.tile([128, 16 * 128], f32, tag=f"kbf{e}",
                               name=f"kbf{e}") for e in range(2)]
                kf = [sp.tile([128, 16 * 128], f32, tag=f"kf{e}",
                              name=f"kf{e}") for e in range(2)]
                kb16 = [sp.tile([128, 16 * 128], bf16, tag=f"kb16_{e}",
                                name=f"kb16_{e}") for e in range(2)]
                k16 = [sp.tile([128, 16 * 128], bf16, tag=f"k16_{e}",
                               name=f"k16_{e}") for e in range(2)]
                kbT = [sp.tile([128, L], bf16, tag=f"kbT{e}", name=f"kbT{e}")
                       for e in range(2)]
                kT = [sp.tile([128, L], bf16, tag=f"kT{e}", name=f"kT{e}")
                      for e in range(2)]
                dvkb = kb_d.rearrange("(c p) e -> p c e", p=128)
                dvk = k_d.rearrange("(c p) e -> p c e", p=128)
                # loads on scalar (kb) / gpsimd (k) queues; casts chunked
                for ec in range(2):
                    tvb = kbf[ec].rearrange("p (c f) -> p c f", c=16)
                    tvk = kf[ec].rearrange("p (c f) -> p c f", c=16)
                    for cq in range(2):
                        cs = slice(cq * 8, (cq + 1) * 8)
                        nc.scalar.dma_start(
                            out=tvb[:, cs, :],
                            in_=dvkb[:, cs, ec * 128:(ec + 1) * 128])
                        nc.gpsimd.dma_start(
                            out=tvk[:, cs, :],
                            in_=dvk[:, cs, ec * 128:(ec + 1) * 128])
                        co = slice(cq * 1024, (cq + 1) * 1024)
                        nc.vector.tensor_copy(kb16[ec][:, co], kbf[ec][:, co])
                        nc.vector.tensor_copy(k16[ec][:, co], kf[ec][:, co])
                vraw = sp.tile([128, 16 * H], f32, tag="vraw")
                vv = vraw.rearrange("p (c e) -> p c e", c=16)
                dvv = v_d.rearrange("(c p) e -> p c e", p=128)
                for c4 in range(2):
                    nc.sync.dma_start(out=vv[:, c4 * 8:(c4 + 1) * 8, :],
                                      in_=dvv[:, c4 * 8:(c4 + 1) * 8, :])
                m16 = sp.tile([16, 128], i32, tag="m16")
                nc.sync.dma_start(out=m16,
                                  in_=mask_d.rearrange("(c p) -> c p", p=128))
                wwraw = sp.tile([128, 2 * H], f32, tag="wwraw")
                nc.sync.dma_start(out=wwraw.rearrange("p (c e) -> p c e", c=2),
                                  in_=Ww_d.rearrange("(c p) e -> p c e", p=128))
                # xbar transposes issued after all independent sync loads
                # (avoids head-of-line blocking on the sync DMA queue)
                for ec in range(2):
                    for cq in range(2):
                        co = slice(cq * 1024, (cq + 1) * 1024)
                        nc.sync.dma_start_transpose(
                            out=kbT[ec].rearrange("p (c f) -> p c f",
                                                  c=16)[:, cq * 8:(cq + 1) * 8, :],
                            in_=kb16[ec][:, co])
                        nc.scalar.dma_start_transpose(
                            out=kT[ec].rearrange("p (c f) -> p c f",
                                                 c=16)[:, cq * 8:(cq + 1) * 8, :],
                            in_=k16[ec][:, co])
                bbt = sp.tile([1, H], f32, tag="bbt")
                nc.sync.dma_start(out=bbt, in_=bb_d[None, :])
                bbb = sp.tile([1, H], bf16, tag="bbb")
                nc.vector.tensor_copy(bbb, bbt)
                bwt = sp.tile([1, H], f32, tag="bwt")
                nc.sync.dma_start(out=bwt, in_=bw_d[None, :])
                # bwrep: bw broadcast to 128 partitions via doubling DMAs
                nc.gpsimd.dma_start(out=bwrep[0:1], in_=bwt)
                for db in range(7):
                    n = 1 << db
                    nc.gpsimd.dma_start(out=bwrep[n:2 * n], in_=bwrep[0:n])
                oneslb = sp.tile([1, L], bf16, tag="oneslb")
                nc.vector.memset(oneslb, 1.0)
                m16f = sp.tile([16, 128], f32, tag="m16f")
                nc.vector.tensor_copy(m16f, m16)
                maskf = sp.tile([128, 16], f32, tag="maskf")
                WbT = [sp.tile([128, H], bf16, tag=f"WbT{e}", name=f"WbT{e}")
                       for e in range(2)]

                # ---------------- prep: transposes & keff ----------------
                with (
                    tc.tile_pool(name="ptr", bufs=4, space="PSUM") as ptr,
                    tc.tile_pool(name="pkeff", bufs=1, space="PSUM") as pkf,
                ):
                    # mask -> maskf [128, 16]
                    tm = ptr.tile([128, 16], f32, tag="tr")
                    nc.tensor.transpose(tm, m16f, ident[0:16, 0:16])
                    nc.vector.tensor_copy(maskf, tm)

                    # vmm: [m|v]/[v|m] layout; v and m both mask-scaled.
                    # v-cols on DVE, m-cols on ACT; split by key-chunk half
                    # so the second vraw load chunk overlaps the first's build
                    vmm6 = vmm.rearrange("p (c hp par two d) -> p c hp par two d",
                                         c=16, hp=4, par=2, two=2)
                    vraw4 = vraw.rearrange("p (c hp par d) -> p c hp par d",
                                           c=16, hp=4, par=2)
                    for chalf in range(2):
                        cs = slice(chalf * 8, (chalf + 1) * 8)
                        for par in range(2):
                            for hp in range(4):
                                nc.vector.scalar_tensor_tensor(
                                    out=vmm6[:, cs, hp, par, 1 - par, :],
                                    in0=vraw4[:, cs, hp, par, :], scalar=1.0,
                                    in1=maskf[:, cs, None].broadcast_to(
                                        [128, 8, 32]),
                                    op0=Alu.mult, op1=Alu.mult)
                                nc.scalar.copy(
                                    vmm6[:, cs, hp, par, par, :],
                                    maskf[:, cs, None].broadcast_to(
                                        [128, 8, 32]))

                    # scale_w slices -> sc8 [128, 8 per q-chunk]
                    for mq in range(8):
                        t = ptr.tile([128, 8], f32, tag="tr", name="t")
                        nc.tensor.transpose(t, swt[:, mq * 128:(mq + 1) * 128],
                                            ident[0:NH, 0:NH])
                        nc.vector.tensor_copy(sc8[:, mq * 8:(mq + 1) * 8], t)

                    # Wb transposes
                    for dc in range(2):
                        for ec in range(2):
                            t = ptr.tile([128, 128], f32, tag="tr", name="t")
                            nc.tensor.transpose(
                                t,
                                wbraw[:, dc * H + ec * 128: dc * H + (ec + 1) * 128],
                                ident)
                            pcopy(WbT[ec][:, dc * 128:(dc + 1) * 128], t)

                    # q: scale by scale_w/sqrt(DH) (DVE)
                    for mq in range(8):
                        qv = qraw[:, mq * H:(mq + 1) * H].rearrange(
                            "p (h j) -> p h j", h=NH)
                        nc.vector.scalar_tensor_tensor(
                            out=qv, in0=qv, scalar=ISQ,
                            in1=sc8[:, mq * 8:(mq + 1) * 8][:, :, None].broadcast_to(
                                [128, 8, 32]),
                            op0=Alu.mult, op1=Alu.mult)

                    def keff_mms(dc, pk):
                        for ec in range(2):
                            for ns in range(4):
                                nc.tensor.matmul(
                                    pk[:, ns * 512:(ns + 1) * 512],
                                    lhsT=WbT[ec][:, dc * 128:(dc + 1) * 128],
                                    rhs=kbT[ec][:, ns * 512:(ns + 1) * 512],
                                    start=(ec == 0), stop=False)
                        for ns in range(4):
                            nc.tensor.matmul(
                                pk[:, ns * 512:(ns + 1) * 512],
                                lhsT=bbb[0:1, dc * 128:(dc + 1) * 128],
                                rhs=oneslb[0:1, ns * 512:(ns + 1) * 512],
                                start=False, stop=True)
                        # evacuate with fused +k add; then 64-row-shifted copy
                        for nh2 in range(2):
                            co = slice(nh2 * 1024, (nh2 + 1) * 1024)
                            nc.vector.tensor_add(keffT[dc][:, co], pk[:, co],
                                                 kT[dc][:, co])
                        nc.sync.dma_start(out=keffT2[dc][0:64],
                                          in_=keffT[dc][64:128])
                        nc.sync.dma_start(out=keffT2[dc][64:128],
                                          in_=keffT[dc][0:64])

                    pk0 = pkf.tile([128, L], f32, tag="pk", name="pk0")
                    keff_mms(0, pk0)

                    # q transposes into qsT
                    for dc in range(2):
                        for mq in range(8):
                            t = ptr.tile([128, 128], f32, tag="tr", name="t")
                            nc.tensor.transpose(
                                t,
                                qraw[:, mq * H + dc * 128: mq * H + (dc + 1) * 128],
                                ident)
                            pcopy(qsT[dc][:, mq * 128:(mq + 1) * 128], t)
                        nc.sync.dma_start(out=qsT2[dc][0:64],
                                          in_=qsT[dc][64:128])
                        nc.sync.dma_start(out=qsT2[dc][64:128],
                                          in_=qsT[dc][0:64])

                    pk1 = pkf.tile([128, L], f32, tag="pk", name="pk1")
                    keff_mms(1, pk1)

                    # Ww transposes (only needed at the end)
                    for er in range(2):
                        for g in range(2):
                            t = ptr.tile([128, 128], f32, tag="tr", name="t")
                            nc.tensor.transpose(
                                t,
                                wwraw[:, er * H + g * 128: er * H + (g + 1) * 128],
                                ident)
                            pcopy(WwT[g][:, er * 128:(er + 1) * 128], t)

            # ---------------- main attention loop ----------------
            # group g: heads (2g, 2g+1); chunk ch = g//2.
            # kc processed in pairs with alternating PE row groups (via the
            # 64-row-shifted tile copies): the pair's 4 QK matmuls occupy 4
            # distinct 32-row groups and stream concurrently.
            with (
                tc.tile_pool(name="pst", bufs=3, space="PSUM") as pst,
                tc.tile_pool(name="ppv", bufs=2, space="PSUM") as ppv,
            ):
                for g in range(4):
                    ch = g // 2
                    pv = [ppv.tile([128, 512], f32, tag="pv",
                                   name=f"pv{g}_{qb}") for qb in range(2)]
                    for kcp in range(8):
                        kcs = (2 * kcp, 2 * kcp + 1)
                        for qb in range(2):
                            sts2 = {}
                            for kc2 in kcs:
                                sts2[kc2] = pst.tile([128, 1024], f32,
                                                     tag="st", name=f"st{kc2 % 2}")
                            for kc2 in kcs:
                                par = kc2 % 2
                                kket = keffT[ch] if par == 0 else keffT2[ch]
                                qqt = qsT[ch] if par == 0 else qsT2[ch]
                                rbase = (g % 2) * 64 if par == 0 else (1 - g % 2) * 64
                                for t in range(2):
                                    ro = rbase + t * 32
                                    nc.tensor.matmul(
                                        sts2[kc2][:, t * 512:(t + 1) * 512],
                                        lhsT=kket[ro:ro + 32,
                                                  kc2 * 128:(kc2 + 1) * 128],
                                        rhs=qqt[ro:ro + 32,
                                                qb * 512:(qb + 1) * 512],
                                        tile_position=(ro, 0),
                                        start=True, stop=True)
                            # exp: qb0 -> ACT; qb1 -> DVE (Schraudolph),
                            # except every DVE_EXCL-th tile -> ACT (balance)
                            pts = {}
                            for kc2 in kcs:
                                io = (g * 16 + kc2) * 2 + qb
                                if kc2 == kcs[0] or io % DVE_EXCL == 1:
                                    pt = ptp.tile([128, 1024], bf16, tag="ptA",
                                                  name="ptA")
                                    nc.scalar.activation(pt, sts2[kc2], Exp)
                                    pts[kc2] = pt
                                else:
                                    pti = ptp.tile([128, 1024], i16, tag="ptD",
                                                   name="ptD")
                                    nc.vector.tensor_scalar(
                                        out=pti, in0=sts2[kc2], scalar1=A16,
                                        scalar2=B16, op0=Alu.mult, op1=Alu.add)
                                    pts[kc2] = pti.bitcast(bf16)
                            # PV: h even lhsT=[m|v] -> rows [den|O];
                            #     h odd  lhsT=[v|m] -> rows [O|den]
                            for kc2 in kcs:
                                for t in range(2):
                                    h = 2 * g + t
                                    nc.tensor.matmul(
                                        pv[qb][64 * t:64 * t + 64, :],
                                        lhsT=vmm[:, (kc2 * NH + h) * 64:
                                                 (kc2 * NH + h) * 64 + 64],
                                        rhs=pts[kc2][:, t * 512:(t + 1) * 512],
                                        tile_position=(0, 64 * t),
                                        start=(kc2 == 0), stop=(kc2 == 15))
                    # normalize: pv rows = [den0 | O0 | O1 | den1].  Full
                    # 128-partition ops (custom DVE ops misbehave at nonzero
                    # partition base); unused lanes compute garbage, unread.
                    for qb in range(2):
                        ntmp = smp.tile([128, 512], f32, tag="ntmp", name="ntmp")
                        nc.vector.reciprocal_approx_fast(ntmp, pv[qb])
                        rtl = smp.tile([128, 512], f32, tag="rtl", name="rtl")
                        nc.sync.dma_start(out=rtl[32:64], in_=ntmp[0:32])
                        nc.sync.dma_start(out=rtl[64:96], in_=ntmp[96:128])
                        hst = smp.tile([128, 512], bf16, tag="hst", name="hst")
                        nc.vector.tensor_mul(hst, pv[qb], rtl)
                        ro2 = (g % 2) * 64
                        nc.sync.dma_start(
                            out=hidT[ch][ro2:ro2 + 64,
                                         qb * 512:(qb + 1) * 512],
                            in_=hst[32:96])

            # ---------------- output linear ----------------
            with tc.tile_pool(name="pout", bufs=2, space="PSUM") as pout:
                for mq in range(8):
                    po = pout.tile([128, H], f32, tag="po", name="po")
                    for gg in range(2):
                        nc.tensor.matmul(
                            po,
                            lhsT=hidT[gg][:, mq * 128:(mq + 1) * 128],
                            rhs=WwT[gg],
                            start=(gg == 0), stop=(gg == 1))
                    nc.vector.tensor_add(outsb[:, mq * H:(mq + 1) * H],
                                          po, bwrep)
                nc.sync.dma_start(
                    out=out_d.rearrange("(c p) e -> p c e", p=128),
                    in_=outsb.rearrange("p (c e) -> p c e", c=8))

    nc.compile()
    return nc


def _make_in_maps(inputs):
    q = np.ascontiguousarray(np.asarray(inputs["q"], dtype=np.float32))
    k = np.ascontiguousarray(np.asarray(inputs["k"], dtype=np.float32))
    v = np.ascontiguousarray(np.asarray(inputs["v"], dtype=np.float32))
    k_b = np.ascontiguousarray(np.asarray(inputs["k_b"], dtype=np.float32))
    mask = np.ascontiguousarray(np.asarray(inputs["mask"], dtype=np.int32))
    sw = np.ascontiguousarray(np.asarray(inputs["scale_w"], dtype=np.float32))
    Wb = np.ascontiguousarray(np.asarray(inputs["Wb"], dtype=np.float32))
    bb = np.ascontiguousarray(np.asarray(inputs["bb"], dtype=np.float32))
    Ww = np.ascontiguousarray(np.asarray(inputs["Ww"], dtype=np.float32))
    bw = np.ascontiguousarray(np.asarray(inputs["bw"], dtype=np.float32))
    ident = np.eye(128, dtype=np.float32)
    in_maps = []
    for c in range(NCORES):
        b, qs = c // 2, c % 2
        in_maps.append({
            "q_s": q[b, qs * LQ:(qs + 1) * LQ, :],
            "k_s": k[b],
            "v_s": v[b],
            "kb_s": k_b[b],
            "mask_s": mask[b],
            "sw_s": np.ascontiguousarray(sw[:, qs * LQ:(qs + 1) * LQ]),
            "Wb": Wb, "bb": bb, "Ww": Ww, "bw": bw,
            "ident": ident,
        })
    return in_maps


def _enable_ldw_opt():
    """Rewrite walrus's hardcoded --enable-ldw-opt=false: identical
    back-to-back weight loads are elided, keeping the PE stream dense."""
    from concourse import bass_utils as bu

    if getattr(bu, "_ldw_patched", False):
        return
    orig = bu.run_command

    def patched(argv, **kwargs):
        # --enable-ldw-opt=true breaks codegen for tile-positioned
        # LDWEIGHTS ("InstLdweights is not compatible with LDW
        # optimization"); leave the flag alone.
        return orig(argv, **kwargs)

    bu.run_command = patched
    bu._ldw_patched = True


def run_sharded(inputs, trace=False, tmpdir=None):
    from concourse import bass_utils
    from concourse.bass_utils import run_bass_kernel_spmd

    _enable_ldw_opt()
    if trace:
        _install_ntff_hook()
        bass_utils.upload_artifacts = lambda d: d
    nc = _build()
    in_maps = _make_in_maps(inputs)
    res = run_bass_kernel_spmd(nc, in_maps, list(range(NCORES)),
                               trace=trace, tmpdir=tmpdir)
    out = np.empty((B, L, H), dtype=np.float32)
    for c in range(NCORES):
        b, qs = c // 2, c % 2
        out[b, qs * LQ:(qs + 1) * LQ, :] = res.results[c]["out"]
    return out, res


def kernel(**inputs):
    out, _ = run_sharded(inputs, trace=False)
    return out


def _install_ntff_hook():
    """Provide antenv.axon_hooks (absent in this image) so trace=True works."""
    import contextlib
    import ctypes
    import types

    import antenv

    if hasattr(antenv, "axon_hooks"):
        return
    mod = types.ModuleType("antenv.axon_hooks")
    _hook = [None]
    mod.set_axon_ntff_profile_hook = lambda h: _hook.__setitem__(0, h)
    mod.get_axon_ntff_profile_hook = lambda: _hook[0]
    antenv.axon_hooks = mod
    sys.modules["antenv.axon_hooks"] = mod

    lib = ctypes.CDLL("/opt/axon/libaxon_pjrt.so")
    if not hasattr(lib, "axon_start_nrt_profile"):
        return
    lib.axon_start_nrt_profile.argtypes = [ctypes.POINTER(ctypes.c_int64),
                                           ctypes.c_size_t]
    lib.axon_start_nrt_profile.restype = ctypes.c_int64
    lib.axon_stop_nrt_profile.argtypes = [ctypes.c_char_p]
    lib.axon_stop_nrt_profile.restype = ctypes.c_int64

    @contextlib.contextmanager
    def _profile(output_dir, device_ids):
        import jax

        jax.devices()
        if device_ids:
            ids = (ctypes.c_int64 * len(device_ids))(*device_ids)
            rc = lib.axon_start_nrt_profile(ids, len(device_ids))
        else:
            rc = lib.axon_start_nrt_profile(None, 0)
        if rc != 0:
            raise RuntimeError(f"axon_start_nrt_profile rc={rc}")
        try:
            yield
        finally:
            n = lib.axon_stop_nrt_profile(str(output_dir).encode())
            print(f"profile: {n} file(s) written to {output_dir}",
                  file=sys.stderr)

    mod.set_axon_ntff_profile_hook(_profile)


# revision 12
# speedup vs baseline: 1.1464x; 1.1273x over previous
"""Trainium2 Bass kernel for AuxiliaryMultiHeadedAttention.

Reference computation (B=4, L=2048, H=256, NH=8, DH=32):
    kb   = split_heads(k_b @ Wb.T + bb)
    corr = (qh @ kh^T + qh @ kb^T) / sqrt(DH) * scale_w[h, q]
    corr = where(mask==0, -1e9, corr);  prob = softmax(corr)
    out  = merge_heads(prob @ vh) @ Ww.T + bw

Kernel strategy (8 NeuronCores):
    Shard (batch, query-half): core c -> batch c//2, queries (c%2)*1024..+1024.
    Each core:
      kT, kbT via bf16 cast + DMA-xbar transpose (no PE transposes)
      keffT = (k + k_b @ Wb.T + bb)^T  [dims, keys]  bf16 (dual QK^T folded)
      qsT   = (q * scale_w/sqrt(DH))^T [dims, queries] bf16
      S^T   = keffT_h^T @ qsT_h  (bf16 MMs, 2 heads row-tiled, fp32 psum;
              row groups alternate with kc parity via 64-row-shifted tile
              copies so LDWEIGHTS overlaps the previous chunk's matmuls)
      P^T   = exp(S^T): split between ACT (exact exp, bf16 out) and DVE
              (Schraudolph: int16(A*x+B) bitcast to bf16, one tensor_scalar)
      PV with lhsT [m|v_h] / [v_h|m] (m = mask: masks both numerator and
              denominator) -> psum rows [den0|O0|O1|den1]
      hidT  = O * recip(den)  (full-partition recip + mul, DMA realign)
      out   = hidT^T @ WwT + bw  (bf16 MMs)
    Host concatenates the 8 [1024, 256] slices.
"""

import sys

if "/opt/trn_rl_repo" not in sys.path:
    sys.path.insert(0, "/opt/trn_rl_repo")

import math

import numpy as np

B, L, H, NH, DH = 4, 2048, 256, 8, 32
LQ = 1024  # queries per core
NCORES = 8
ISQ = 1.0 / math.sqrt(DH)

# Schraudolph exp for bf16 target: bf16bits(exp(x)) ~ int16(A16*x + B16)
A16 = 128.0 / math.log(2.0)
C_OFF = 5.5
B16 = 127.0 * 128.0 - C_OFF
# every DVE_EXCL-th qb1 exp tile goes to ACT instead of DVE (load balance)
DVE_EXCL = 6


def _build():
    import concourse.bass as bass  # noqa: F401
    import concourse.mybir as mybir
    import concourse.tile as tile
    from concourse import bacc

    f32 = mybir.dt.float32
    i32 = mybir.dt.int32
    i16 = mybir.dt.int16
    bf16 = mybir.dt.bfloat16
    Exp = mybir.ActivationFunctionType.Exp
    Alu = mybir.AluOpType

    nc = bacc.Bacc("TRN2", target_bir_lowering=False, debug=False, num_devices=NCORES)

    q_d = nc.dram_tensor("q_s", [LQ, H], f32, kind="ExternalInput")
    k_d = nc.dram_tensor("k_s", [L, H], f32, kind="ExternalInput")
    v_d = nc.dram_tensor("v_s", [L, H], f32, kind="ExternalInput")
    kb_d = nc.dram_tensor("kb_s", [L, H], f32, kind="ExternalInput")
    mask_d = nc.dram_tensor("mask_s", [L], i32, kind="ExternalInput")
    sw_d = nc.dram_tensor("sw_s", [NH, LQ], f32, kind="ExternalInput")
    Wb_d = nc.dram_tensor("Wb", [H, H], f32, kind="ExternalInput")
    bb_d = nc.dram_tensor("bb", [H], f32, kind="ExternalInput")
    Ww_d = nc.dram_tensor("Ww", [H, H], f32, kind="ExternalInput")
    bw_d = nc.dram_tensor("bw", [H], f32, kind="ExternalInput")
    id_d = nc.dram_tensor("ident", [128, 128], f32, kind="ExternalInput")
    out_d = nc.dram_tensor("out", [LQ, H], f32, kind="ExternalOutput")

    copy_flip = [0]

    with tile.TileContext(nc) as tc:
        with (
            tc.tile_pool(name="persist", bufs=1) as pp,
            tc.tile_pool(name="pt", bufs=3) as ptp,
            tc.tile_pool(name="small", bufs=2) as smp,
        ):
            # ---------------- persistent SBUF tensors ----------------
            ident = pp.tile([128, 128], f32, tag="ident")
            nc.sync.dma_start(out=ident, in_=id_d[:, :])
            keffT = [pp.tile([128, L], bf16, tag=f"keffT{g}", name=f"keffT{g}")
                     for g in range(2)]
            keffT2 = [pp.tile([128, L], bf16, tag=f"keffT2_{g}",
                              name=f"keffT2_{g}") for g in range(2)]
            qsT = [pp.tile([128, LQ], bf16, tag=f"qsT{g}", name=f"qsT{g}")
                   for g in range(2)]
            qsT2 = [pp.tile([128, LQ], bf16, tag=f"qsT2_{g}", name=f"qsT2_{g}")
                    for g in range(2)]
            # per (key-chunk, head): [m|v_h] (h even) / [v_h|m] (h odd);
            # m = mask column (masks numerator and denominator)
            vmm = pp.tile([128, 16 * NH * 64], bf16, tag="vmm")
            hidT = [pp.tile([128, LQ], bf16, tag=f"hidT{g}", name=f"hidT{g}")
                    for g in range(2)]
            WwT = [pp.tile([128, H], bf16, tag=f"WwT{g}", name=f"WwT{g}")
                   for g in range(2)]
            bwrep = pp.tile([128, H], f32, tag="bwrep")
            sc8 = pp.tile([128, 64], f32, tag="sc8")
            outsb = pp.tile([128, 8 * H], f32, tag="outsb")

            with tc.tile_pool(name="stage", bufs=1) as sp:
                def pcopy(dst, src):
                    # alternate psum->sbuf evacuation between DVE and ACT
                    if copy_flip[0] % 2 == 0:
                        nc.vector.tensor_copy(dst, src)
                    else:
                        nc.scalar.copy(dst, src)
                    copy_flip[0] += 1

                # ---------------- staging loads ----------------
                # queue discipline: sync = small/critical loads in priority
                # order; gpsimd = bulk k/k_b chunks + bwrep; scalar = xbar
                # transposes then ACT compute.  Each queue's order matches
                # criticality (sem-count waits make queue order = deps).
                m16 = sp.tile([16, 128], i32, tag="m16")
                nc.sync.dma_start(out=m16,
                                  in_=mask_d.rearrange("(c p) -> c p", p=128))
                swt = sp.tile([NH, LQ], f32, tag="swt")
                nc.sync.dma_start(out=swt, in_=sw_d[:, :])
                qraw = sp.tile([128, 8 * H], f32, tag="qraw")
                nc.sync.dma_start(out=qraw.rearrange("p (c e) -> p c e", c=8),
                                  in_=q_d.rearrange("(c p) e -> p c e", p=128))
                wbraw = sp.tile([128, 2 * H], f32, tag="wbraw")
                nc.sync.dma_start(out=wbraw.rearrange("p (c e) -> p c e", c=2),
                                  in_=Wb_d.rearrange("(c p) e -> p c e", p=128))
                vraw = sp.tile([128, 16 * H], f32, tag="vraw")
                vv = vraw.rearrange("p (c e) -> p c e", c=16)
                dvv = v_d.rearrange("(c p) e -> p c e", p=128)
                for c4 in range(2):
                    nc.sync.dma_start(out=vv[:, c4 * 8:(c4 + 1) * 8, :],
                                      in_=dvv[:, c4 * 8:(c4 + 1) * 8, :])
                bbt = sp.tile([1, H], f32, tag="bbt")
                nc.sync.dma_start(out=bbt, in_=bb_d[None, :])
                bwt = sp.tile([1, H], f32, tag="bwt")
                nc.sync.dma_start(out=bwt, in_=bw_d[None, :])
                wwraw = sp.tile([128, 2 * H], f32, tag="wwraw")
                nc.sync.dma_start(out=wwraw.rearrange("p (c e) -> p c e", c=2),
                                  in_=Ww_d.rearrange("(c p) e -> p c e", p=128))

                # bulk k/k_b on the gpsimd queue (runs parallel to sync)
                kbf = [sp.tile([128, 16 * 128], f32, tag=f"kbf{e}",
                               name=f"kbf{e}") for e in range(2)]
                kf = [sp.tile([128, 16 * 128], f32, tag=f"kf{e}",
                              name=f"kf{e}") for e in range(2)]
                dvkb = kb_d.rearrange("(c p) e -> p c e", p=128)
                dvk = k_d.rearrange("(c p) e -> p c e", p=128)
                for tiles, dram in ((kbf, dvkb), (kf, dvk)):
                    for ec in range(2):
                        tv = tiles[ec].rearrange("p (c f) -> p c f", c=16)
                        for cq in range(2):
                            cs = slice(cq * 8, (cq + 1) * 8)
                            nc.gpsimd.dma_start(
                                out=tv[:, cs, :],
                                in_=dram[:, cs, ec * 128:(ec + 1) * 128])
                # bwrep: bw broadcast to 128 partitions via doubling DMAs
                nc.gpsimd.dma_start(out=bwrep[0:1], in_=bwt)
                for db in range(7):
                    n = 1 << db
                    nc.gpsimd.dma_start(out=bwrep[n:2 * n], in_=bwrep[0:n])

                kb16 = [sp.tile([128, 16 * 128], bf16, tag=f"kb16_{e}",
                                name=f"kb16_{e}") for e in range(2)]
                k16 = [sp.tile([128, 16 * 128], bf16, tag=f"k16_{e}",
                               name=f"k16_{e}") for e in range(2)]
                kbT = [sp.tile([128, L], bf16, tag=f"kbT{e}", name=f"kbT{e}")
                       for e in range(2)]
                kT = [sp.tile([128, L], bf16, tag=f"kT{e}", name=f"kT{e}")
                      for e in range(2)]
                # DVE: casts first (critical for keff), then mask copy
                for ec in range(2):
                    nc.vector.tensor_copy(kb16[ec], kbf[ec])
                for ec in range(2):
                    nc.vector.tensor_copy(k16[ec], kf[ec])
                m16f = sp.tile([16, 128], f32, tag="m16f")
                nc.vector.tensor_copy(m16f, m16)
                bbb = sp.tile([1, H], bf16, tag="bbb")
                nc.vector.tensor_copy(bbb, bbt)
                oneslb = sp.tile([1, L], bf16, tag="oneslb")
                nc.vector.memset(oneslb, 1.0)
                # scalar queue: xbar transposes (then ACT compute follows)
                for ec in range(2):
                    nc.scalar.dma_start_transpose(
                        out=kbT[ec].rearrange("p (c f) -> p c f", c=16),
                        in_=kb16[ec])
                for ec in range(2):
                    nc.scalar.dma_start_transpose(
                        out=kT[ec].rearrange("p (c f) -> p c f", c=16),
                        in_=k16[ec])
                maskf = sp.tile([128, 16], f32, tag="maskf")
                WbT = [sp.tile([128, H], bf16, tag=f"WbT{e}", name=f"WbT{e}")
                       for e in range(2)]

                # ---------------- prep: transposes & keff ----------------
                with (
                    tc.tile_pool(name="ptr", bufs=4, space="PSUM") as ptr,
                    tc.tile_pool(name="pkeff", bufs=1, space="PSUM") as pkf,
                ):
                    # PE queue: sw -> Wb -> mask -> q -> keff0 -> keff1 -> Ww
                    # scale_w slices -> sc8 [128, 8 per q-chunk]
                    for mq in range(8):
                        t = ptr.tile([128, 8], f32, tag="tr", name="t")
                        nc.tensor.transpose(t, swt[:, mq * 128:(mq + 1) * 128],
                                            ident[0:NH, 0:NH])
                        nc.vector.tensor_copy(sc8[:, mq * 8:(mq + 1) * 8], t)

                    # Wb transposes
                    for dc in range(2):
                        for ec in range(2):
                            t = ptr.tile([128, 128], f32, tag="tr", name="t")
                            nc.tensor.transpose(
                                t,
                                wbraw[:, dc * H + ec * 128: dc * H + (ec + 1) * 128],
                                ident)
                            pcopy(WbT[ec][:, dc * 128:(dc + 1) * 128], t)

                    # mask -> maskf [128, 16]
                    tm = ptr.tile([128, 16], f32, tag="tr")
                    nc.tensor.transpose(tm, m16f, ident[0:16, 0:16])
                    nc.vector.tensor_copy(maskf, tm)

                    # q: scale by scale_w/sqrt(DH) (DVE)
                    for mq in range(8):
                        qv = qraw[:, mq * H:(mq + 1) * H].rearrange(
                            "p (h j) -> p h j", h=NH)
                        nc.vector.scalar_tensor_tensor(
                            out=qv, in0=qv, scalar=ISQ,
                            in1=sc8[:, mq * 8:(mq + 1) * 8][:, :, None].broadcast_to(
                                [128, 8, 32]),
                            op0=Alu.mult, op1=Alu.mult)

                    # q transposes into qsT
                    for dc in range(2):
                        for mq in range(8):
                            t = ptr.tile([128, 128], f32, tag="tr", name="t")
                            nc.tensor.transpose(
                                t,
                                qraw[:, mq * H + dc * 128: mq * H + (dc + 1) * 128],
                                ident)
                            pcopy(qsT[dc][:, mq * 128:(mq + 1) * 128], t)
                        nc.sync.dma_start(out=qsT2[dc][0:64],
                                          in_=qsT[dc][64:128])
                        nc.sync.dma_start(out=qsT2[dc][64:128],
                                          in_=qsT[dc][0:64])

                    def keff_mms(dc, pk):
                        for ec in range(2):
                            for ns in range(4):
                                nc.tensor.matmul(
                                    pk[:, ns * 512:(ns + 1) * 512],
                                    lhsT=WbT[ec][:, dc * 128:(dc + 1) * 128],
                                    rhs=kbT[ec][:, ns * 512:(ns + 1) * 512],
                                    start=(ec == 0), stop=False)
                        for ns in range(4):
                            nc.tensor.matmul(
                                pk[:, ns * 512:(ns + 1) * 512],
                                lhsT=bbb[0:1, dc * 128:(dc + 1) * 128],
                                rhs=oneslb[0:1, ns * 512:(ns + 1) * 512],
                                start=False, stop=True)
                        # evacuate with fused +k add; then 64-row-shifted copy
                        for nh2 in range(2):
                            co = slice(nh2 * 1024, (nh2 + 1) * 1024)
                            nc.vector.tensor_add(keffT[dc][:, co], pk[:, co],
                                                 kT[dc][:, co])
                        nc.sync.dma_start(out=keffT2[dc][0:64],
                                          in_=keffT[dc][64:128])
                        nc.sync.dma_start(out=keffT2[dc][64:128],
                                          in_=keffT[dc][0:64])

                    pk0 = pkf.tile([128, L], f32, tag="pk", name="pk0")
                    keff_mms(0, pk0)

                    # vmm: [m|v]/[v|m]: v-cols on DVE (STT), m-cols on ACT
                    vmm6 = vmm.rearrange("p (c hp par two d) -> p c hp par two d",
                                         c=16, hp=4, par=2, two=2)
                    vraw4 = vraw.rearrange("p (c hp par d) -> p c hp par d",
                                           c=16, hp=4, par=2)
                    for par in range(2):
                        for hp in range(4):
                            nc.vector.scalar_tensor_tensor(
                                out=vmm6[:, :, hp, par, 1 - par, :],
                                in0=vraw4[:, :, hp, par, :], scalar=1.0,
                                in1=maskf[:, :, None].broadcast_to(
                                    [128, 16, 32]),
                                op0=Alu.mult, op1=Alu.mult)
                            nc.scalar.copy(
                                vmm6[:, :, hp, par, par, :],
                                maskf[:, :, None].broadcast_to(
                                    [128, 16, 32]))

                    pk1 = pkf.tile([128, L], f32, tag="pk", name="pk1")
                    keff_mms(1, pk1)

                    # Ww transposes (only needed at the end)
                    for er in range(2):
                        for g in range(2):
                            t = ptr.tile([128, 128], f32, tag="tr", name="t")
                            nc.tensor.transpose(
                                t,
                                wwraw[:, er * H + g * 128: er * H + (g + 1) * 128],
                                ident)
                            pcopy(WwT[g][:, er * 128:(er + 1) * 128], t)

            # ---------------- main attention loop ----------------
            # group g: heads (2g, 2g+1); chunk ch = g//2.
            # kc processed in pairs with alternating PE row groups (via the
            # 64-row-shifted tile copies): the pair's 4 QK matmuls occupy 4
            # distinct 32-row groups and stream concurrently.
            with (
                tc.tile_pool(name="pst", bufs=3, space="PSUM") as pst,
                tc.tile_pool(name="ppv", bufs=2, space="PSUM") as ppv,
            ):
                for g in range(4):
                    ch = g // 2
                    pv = [ppv.tile([128, 512], f32, tag="pv",
                                   name=f"pv{g}_{qb}") for qb in range(2)]
                    for kcp in range(8):
                        kcs = (2 * kcp, 2 * kcp + 1)
                        for qb in range(2):
                            sts2 = {}
                            for kc2 in kcs:
                                sts2[kc2] = pst.tile([128, 1024], f32,
                                                     tag="st", name=f"st{kc2 % 2}")
                            for kc2 in kcs:
                                par = kc2 % 2
                                kket = keffT[ch] if par == 0 else keffT2[ch]
                                qqt = qsT[ch] if par == 0 else qsT2[ch]
                                rbase = (g % 2) * 64 if par == 0 else (1 - g % 2) * 64
                                for t in range(2):
                                    ro = rbase + t * 32
                                    nc.tensor.matmul(
                                        sts2[kc2][:, t * 512:(t + 1) * 512],
                                        lhsT=kket[ro:ro + 32,
                                                  kc2 * 128:(kc2 + 1) * 128],
                                        rhs=qqt[ro:ro + 32,
                                                qb * 512:(qb + 1) * 512],
                                        tile_position=(ro, 0),
                                        start=True, stop=True)
                            # exp: qb0 -> ACT; qb1 -> DVE (Schraudolph),
                            # except every DVE_EXCL-th tile -> ACT (balance)
                            pts = {}
                            for kc2 in kcs:
                                io = (g * 16 + kc2) * 2 + qb
                                if kc2 == kcs[0] or io % DVE_EXCL == 1:
                                    pt = ptp.tile([128, 1024], bf16, tag="ptA",
                                                  name="ptA")
                                    nc.scalar.activation(pt, sts2[kc2], Exp)
                                    pts[kc2] = pt
                                else:
                                    pti = ptp.tile([128, 1024], i16, tag="ptD",
                                                   name="ptD")
                                    nc.vector.tensor_scalar(
                                        out=pti, in0=sts2[kc2], scalar1=A16,
                                        scalar2=B16, op0=Alu.mult, op1=Alu.add)
                                    pts[kc2] = pti.bitcast(bf16)
                            # PV: h even lhsT=[m|v] -> rows [den|O];
                            #     h odd  lhsT=[v|m] -> rows [O|den]
                            for kc2 in kcs:
                                for t in range(2):
                                    h = 2 * g + t
                                    nc.tensor.matmul(
                                        pv[qb][64 * t:64 * t + 64, :],
                                        lhsT=vmm[:, (kc2 * NH + h) * 64:
                                                 (kc2 * NH + h) * 64 + 64],
                                        rhs=pts[kc2][:, t * 512:(t + 1) * 512],
                                        tile_position=(0, 64 * t),
                                        start=(kc2 == 0), stop=(kc2 == 15))
                    # normalize: pv rows = [den0 | O0 | O1 | den1].  Full
                    # 128-partition ops (custom DVE ops misbehave at nonzero
                    # partition base); unused lanes compute garbage, unread.
                    for qb in range(2):
                        ntmp = smp.tile([128, 512], f32, tag="ntmp", name="ntmp")
                        nc.vector.reciprocal_approx_fast(ntmp, pv[qb])
                        rtl = smp.tile([128, 512], f32, tag="rtl", name="rtl")
                        nc.sync.dma_start(out=rtl[32:64], in_=ntmp[0:32])
                        nc.sync.dma_start(out=rtl[64:96], in_=ntmp[96:128])
                        hst = smp.tile([128, 512], bf16, tag="hst", name="hst")
                        nc.vector.tensor_mul(hst, pv[qb], rtl)
                        ro2 = (g % 2) * 64
                        nc.sync.dma_start(
                            out=hidT[ch][ro2:ro2 + 64,
                                         qb * 512:(qb + 1) * 512],
                            in_=hst[32:96])

            # ---------------- output linear ----------------
            with tc.tile_pool(name="pout", bufs=2, space="PSUM") as pout:
                for mq in range(8):
                    po = pout.tile([128, H], f32, tag="po", name="po")
                    for gg in range(2):
                        nc.tensor.matmul(
                            po,
                            lhsT=hidT[gg][:, mq * 128:(mq + 1) * 128],
                            rhs=WwT[gg],
                            start=(gg == 0), stop=(gg == 1))
                    nc.vector.tensor_add(outsb[:, mq * H:(mq + 1) * H],
                                          po, bwrep)
                nc.sync.dma_start(
                    out=out_d.rearrange("(c p) e -> p c e", p=128),
                    in_=outsb.rearrange("p (c e) -> p c e", c=8))

    nc.compile()
    return nc


def _make_in_maps(inputs):
    q = np.ascontiguousarray(np.asarray(inputs["q"], dtype=np.float32))
    k = np.ascontiguousarray(np.asarray(inputs["k"], dtype=np.float32))
    v = np.ascontiguousarray(np.asarray(inputs["v"], dtype=np.float32))
    k_b = np.ascontiguousarray(np.asarray(inputs["k_b"], dtype=np.float32))
    mask = np.ascontiguousarray(np.asarray(inputs["mask"], dtype=np.int32))
    sw = np.ascontiguousarray(np.asarray(inputs["scale_w"], dtype=np.float32))
    Wb = np.ascontiguousarray(np.asarray(inputs["Wb"], dtype=np.float32))
    bb = np.ascontiguousarray(np.asarray(inputs["bb"], dtype=np.float32))
    Ww = np.ascontiguousarray(np.asarray(inputs["Ww"], dtype=np.float32))
    bw = np.ascontiguousarray(np.asarray(inputs["bw"], dtype=np.float32))
    ident = np.eye(128, dtype=np.float32)
    in_maps = []
    for c in range(NCORES):
        b, qs = c // 2, c % 2
        in_maps.append({
            "q_s": q[b, qs * LQ:(qs + 1) * LQ, :],
            "k_s": k[b],
            "v_s": v[b],
            "kb_s": k_b[b],
            "mask_s": mask[b],
            "sw_s": np.ascontiguousarray(sw[:, qs * LQ:(qs + 1) * LQ]),
            "Wb": Wb, "bb": bb, "Ww": Ww, "bw": bw,
            "ident": ident,
        })
    return in_maps


def _enable_ldw_opt():
    """Rewrite walrus's hardcoded --enable-ldw-opt=false: identical
    back-to-back weight loads are elided, keeping the PE stream dense."""
    from concourse import bass_utils as bu

    if getattr(bu, "_ldw_patched", False):
        return
    orig = bu.run_command

    def patched(argv, **kwargs):
        # --enable-ldw-opt=true breaks codegen for tile-positioned
        # LDWEIGHTS ("InstLdweights is not compatible with LDW
        # optimization"); leave the flag alone.
        return orig(argv, **kwargs)

    bu.run_command = patched
    bu._ldw_patched = True


def run_sharded(inputs, trace=False, tmpdir=None):
    from concourse import bass_utils
    from concourse.bass_utils import run_bass_kernel_spmd

    _enable_ldw_opt()
    if trace:
        _install_ntff_hook()
        bass_utils.upload_artifacts = lambda d: d
    nc = _build()
    in_maps = _make_in_maps(inputs)
    res = run_bass_kernel_spmd(nc, in_maps, list(range(NCORES)),
                               trace=trace, tmpdir=tmpdir)
    out = np.empty((B, L, H), dtype=np.float32)
    for c in range(NCORES):
        b, qs = c // 2, c % 2
        out[b, qs * LQ:(qs + 1) * LQ, :] = res.results[c]["out"]
    return out, res


def kernel(**inputs):
    out, _ = run_sharded(inputs, trace=False)
    return out


def _install_ntff_hook():
    """Provide antenv.axon_hooks (absent in this image) so trace=True works."""
    import contextlib
    import ctypes
    import types

    import antenv

    if hasattr(antenv, "axon_hooks"):
        return
    mod = types.ModuleType("antenv.axon_hooks")
    _hook = [None]
    mod.set_axon_ntff_profile_hook = lambda h: _hook.__setitem__(0, h)
    mod.get_axon_ntff_profile_hook = lambda: _hook[0]
    antenv.axon_hooks = mod
    sys.modules["antenv.axon_hooks"] = mod

    lib = ctypes.CDLL("/opt/axon/libaxon_pjrt.so")
    if not hasattr(lib, "axon_start_nrt_profile"):
        return
    lib.axon_start_nrt_profile.argtypes = [ctypes.POINTER(ctypes.c_int64),
                                           ctypes.c_size_t]
    lib.axon_start_nrt_profile.restype = ctypes.c_int64
    lib.axon_stop_nrt_profile.argtypes = [ctypes.c_char_p]
    lib.axon_stop_nrt_profile.restype = ctypes.c_int64

    @contextlib.contextmanager
    def _profile(output_dir, device_ids):
        import jax

        jax.devices()
        if device_ids:
            ids = (ctypes.c_int64 * len(device_ids))(*device_ids)
            rc = lib.axon_start_nrt_profile(ids, len(device_ids))
        else:
            rc = lib.axon_start_nrt_profile(None, 0)
        if rc != 0:
            raise RuntimeError(f"axon_start_nrt_profile rc={rc}")
        try:
            yield
        finally:
            n = lib.axon_stop_nrt_profile(str(output_dir).encode())
            print(f"profile: {n} file(s) written to {output_dir}",
                  file=sys.stderr)

    mod.set_axon_ntff_profile_hook(_profile)
